# revision 51
# baseline (speedup 1.0000x reference)
"""Trainium2 Bass kernel for nn_ATT_critic (attention critic network).

Strategy: data-parallel over batch across 8 NeuronCores (1024 rows/core),
2 chunks of 512 rows per core; all big GEMMs on the PE in bf16 (PSUM
accumulation in fp32).

Key design points:
  - host-side weight folding: W_fused = W_dec_in @ W_dh (+ fused bias) is a
    weight-only precompute, done once on the host and shipped augmented with
    its bias as a [97, HID] tensor (ones-row trick).
  - host-side transposes + bf16 cast: s^T and a^T are passed per-core in
    bf16 so the kernel DMAs activations directly in [feature, row] layout;
    all weights are pre-cast to bf16 (the PE's fp32r mode rounds to
    bf16-level precision anyway, so this costs ~nothing numerically and
    halves all weight DMA traffic, which was the p3 bottleneck).
  - bias via PSUM prefill: the heads-layer biases are partition-replicated
    once (GpSimd partition_broadcast) and copied into PSUM before each
    accumulation group (alternating DVE/ACT), replacing 128 K=1 ones-row
    bias matmuls (322 ns each on the PE).
  - p2b bias via K-augmentation (97-row stationary with a host-side ones
    row in aT).
  - emission order overlaps chunk 1's input DMA + enc GEMM with chunk 0's
    softmax/context tail; softmax normalization is interleaved with the
    context transposes per row-tile; the weight pool is quad-buffered so
    upcoming layers' W tiles prefetch during the heads GEMM.
"""

import numpy as np

import concourse.bass as bass
import concourse.tile as tile
from concourse import mybir
from concourse import bacc
from concourse.masks import make_identity

P = 128
B = 8192
NCORES = 8
RPC = B // NCORES        # rows per core
CH = 512                 # rows per chunk
NCHUNK = RPC // CH
MT = CH // P             # row tiles per chunk
HID = 1024
KT = HID // P            # k tiles over hidden dim
NH = 8                   # heads
ACTD = 32
DEC_IN = 96
ENC_REM = 32             # 544 - 512

F32 = mybir.dt.float32
BF16 = mybir.dt.bfloat16
AF = mybir.ActivationFunctionType
ALU = mybir.AluOpType
AX = mybir.AxisListType

F32_WEIGHTS = ["b_enc_in", "b_eh", "b_heads", "b1", "b2"]
BF16_WEIGHTS = ["W_enc_in", "W_eh", "W_heads", "W1", "W2"]


def _body(nc, tc, io, ctx):
    q_ap = io["q"]

    const = ctx.enter_context(tc.tile_pool(name="const", bufs=1))
    acts = ctx.enter_context(tc.tile_pool(name="acts", bufs=1))
    wp = ctx.enter_context(tc.tile_pool(name="wp", bufs=4))
    ps = ctx.enter_context(tc.tile_pool(name="ps", bufs=1, space="PSUM"))

    def wtile(shape, name):
        return wp.tile(shape, BF16, tag="w", bufs=4, name=name)

    def t8tile(shape, name, dtype=BF16):
        return acts.tile(shape, dtype, tag="t8", bufs=3, name=name)

    def junk(shape, dtype, name):
        return acts.tile(shape, dtype, tag="junk", bufs=2, name=name)

    def psmm(name, shape=None):
        return ps.tile(shape or [P, 512], F32, tag="mm", bufs=4, name=name)

    def pstr(name, dtype=F32):
        return ps.tile([P, 512], dtype, tag="tr", bufs=2, name=name)

    def psq(name):
        return ps.tile([1, 512], F32, tag="q", bufs=2, name=name)

    # ---------------- constants / one-time init ----------------
    identity_bf = const.tile([P, P], BF16, name="identity_bf")
    make_identity(nc, identity_bf)

    # [b_enc | b_eh | b1] merged into one DMA (fewer descriptors: descriptor
    # generation throughput is the startup bottleneck)
    bias_pp = const.tile([P, 3 * KT], F32, name="bias_pp")
    nc.sync.dma_start(bias_pp, io["bias_pp"])
    b_enc_pp = bias_pp[:, 0:KT]
    b_eh_pp = bias_pp[:, KT:2 * KT]
    b1_pp = bias_pp[:, 2 * KT:3 * KT]
    W2sb = const.tile([P, KT], BF16, name="W2sb")
    nc.sync.dma_start(W2sb, io["W2"])
    b2sb = const.tile([1, 1], F32, name="b2sb")
    nc.sync.dma_start(b2sb, io["b2"][None, :])
    # fused decoder weights (host-folded), bias as row 96
    wfa_sb = const.tile([DEC_IN + 1, HID], BF16, name="wfa_sb")
    nc.sync.dma_start(wfa_sb, io["Wfa"])

    b_full = const.tile([P, NH, HID], BF16, name="b_full")

    def binit():
        # head biases (host-cast bf16, one DMA to partition 0) replicated
        # across all partitions via GpSimd partition_broadcast
        bh_row = const.tile([1, NH * HID], BF16, name="bh_row")
        nc.sync.dma_start(bh_row, io["b_heads"])
        for h in range(NH):
            nc.gpsimd.partition_broadcast(
                b_full[:, h, :], bh_row[0:1, h * HID:(h + 1) * HID])

    # ---------------- per-chunk stages ----------------
    _scope_stack = [None]

    def sc_(c, nm):
        prev = _scope_stack.pop()
        if prev is not None:
            nc.leave_named_scope(prev[0], prev[1], False)
        if nm is not None:
            full = f"c{c}_{nm}"
            sid, _ = nc.enter_named_scope(full, False)
            _scope_stack.append((full, sid))
        else:
            _scope_stack.append(None)

    S = [dict() for _ in range(NCHUNK)]

    # all startup-critical tensors (s^T tiled, W_enc tiled, W_enc remainder,
    # a^T with its ones row) ride in ONE host-packed DMA: descriptor
    # generation throughput, not bandwidth, limits the startup, so 128 fat
    # descriptors beat ~450 small ones. Loaded once, for both chunks.
    PK_ST, PK_WENC, PK_WENCR, PK_AOWN, PK_AOTH, PKW = (
        0, 4 * RPC, 8 * RPC, 9 * RPC, 10 * RPC, 11 * RPC)
    pk = const.tile([P, PKW], BF16, name="pk")
    nc.sync.dma_start(pk, io["pk"])
    full = dict(
        sT=pk[:, PK_ST:PK_WENC].rearrange("p (kt r) -> p kt r", kt=4),
        wenc=pk[:, PK_WENC:PK_WENCR].rearrange("p (ko f) -> p ko f", ko=4),
        wencr=pk[0:ENC_REM, PK_WENCR:PK_WENCR + HID],
        aownT=pk[0:ACTD, PK_AOWN:PK_AOTH],
        aothT=pk[0:DEC_IN + 1, PK_AOTH:PKW],
    )

    def p1(c):
        st = S[c]
        r0 = c * CH
        st.update(sT=full["sT"][:, :, r0:r0 + CH],
                  aownT=full["aownT"][:, r0:r0 + CH],
                  aothT=full["aothT"][:, r0:r0 + CH])

    def p1b(c):
        st = S[c]
        wenc = full["wenc"]
        enc_inT = t8tile([P, KT, CH], "enc_inT")
        for m in range(KT):
            pm = psmm("pm_enc")
            for kt in range(4):
                nc.tensor.matmul(pm, lhsT=wenc[:, kt, m * P:(m + 1) * P],
                                 rhs=st["sT"][:, kt, :], start=(kt == 0),
                                 stop=False)
            nc.tensor.matmul(pm, lhsT=full["wencr"][:, m * P:(m + 1) * P],
                             rhs=st["aownT"], start=False, stop=True)
            nc.scalar.activation(enc_inT[:, m, :], pm, AF.Identity,
                                 bias=b_enc_pp[:, m:m + 1])
        st["enc_inT"] = enc_inT

    def p2a(c):
        st = S[c]
        encHT = t8tile([P, KT, CH], "encHT")
        for mh in range(2):
            weh = wtile([P, KT, 512], "weh")
            nc.sync.dma_start(
                weh, io["W_eh"][mh].rearrange("p (ko f) -> p ko f", ko=KT))
            for mi in range(4):
                m = mh * 4 + mi
                pm = psmm("pm_eh")
                for kt in range(KT):
                    nc.tensor.matmul(pm, lhsT=weh[:, kt, mi * P:(mi + 1) * P],
                                     rhs=st["enc_inT"][:, kt, :],
                                     start=(kt == 0), stop=(kt == KT - 1))
                nc.scalar.activation(encHT[:, m, :], pm, AF.Relu,
                                     bias=b_eh_pp[:, m:m + 1])
        st["encHT"] = encHT

    def p2b(c):
        # DH = relu([a_others, 1] @ [W_fused; b_fused]) : K=97, no bias matmul
        st = S[c]
        DH = acts.tile([P, MT, HID], BF16, tag="dh", bufs=1, name="DH")
        for mt in range(MT):
            for n in range(2):
                pm = psmm("pm_dh")
                nc.tensor.matmul(pm,
                                 lhsT=st["aothT"][:, mt * P:(mt + 1) * P],
                                 rhs=wfa_sb[:, n * 512:(n + 1) * 512],
                                 start=True, stop=True)
                nc.scalar.activation(DH[:, mt, n * 512:(n + 1) * 512], pm,
                                     AF.Relu)
        st["DH"] = DH

    def p3(c):
        st = S[c]
        EH = acts.tile([P, MT, NH, HID], BF16, tag="eh", bufs=1, name="EH")
        scores = acts.tile([P, MT, NH], F32, tag="scores", bufs=2, name="scores")

        def emit_group(h, n, mt, whn):
            pm = psmm("pm_hd")
            # bias prefill from the replicated tile (alternating DVE/ACT,
            # off the PE critical path); matmuls accumulate on top.
            # GpSimd cannot write PSUM on TRN2.
            if mt == 0:
                nc.vector.tensor_copy(pm, b_full[:, h, n * 512:(n + 1) * 512])
            else:
                nc.scalar.activation(pm, b_full[:, h, n * 512:(n + 1) * 512],
                                     AF.Copy)
            for kt in range(KT):
                nc.tensor.matmul(
                    pm, lhsT=st["encHT"][:, kt, mt * P:(mt + 1) * P],
                    rhs=whn[:, kt, :], start=False,
                    stop=(kt == KT - 1), skip_group_check=True)
            nc.scalar.activation(EH[:, mt, h, n * 512:(n + 1) * 512],
                                 pm, AF.Relu)

        for h in range(NH):
            whns = []
            for n in range(2):
                whn = wtile([P, KT, 512], f"whn{h}_{n}")
                nc.sync.dma_start(
                    whn, io["W_heads"][h, n].rearrange("p (ko f) -> p ko f",
                                                       ko=KT))
                whns.append(whn)
                if h < NH - 1:
                    for mt in range(MT):
                        emit_group(h, n, mt, whn)
            if h >= 1:
                # head h-1's scores are complete: fold its (unnormalized)
                # softmax contribution into the context now, spread evenly
                # across heads so the DVE queue never floods and delays the
                # PSUM bias prefills. exp without max-subtraction is safe:
                # scores ~< 12 here.
                hp = h - 1
                if h == 1:
                    exps = acts.tile([P, MT, NH], F32, tag="attn", bufs=2,
                                     name="exps")
                    ctx_t = t8tile([P, MT, HID], "ctx_t")
                    st["stats"] = acts.tile([P, MT, 2], F32, tag="stats",
                                            bufs=2, name="stats")
                for mt in range(MT):
                    nc.scalar.activation(exps[:, mt, hp:hp + 1],
                                         scores[:, mt, hp:hp + 1], AF.Exp)
                for mt in range(MT):
                    if hp == 0:
                        nc.vector.tensor_scalar_mul(ctx_t[:, mt, :],
                                                    EH[:, mt, 0, :],
                                                    exps[:, mt, 0:1])
                    else:
                        nc.vector.scalar_tensor_tensor(
                            out=ctx_t[:, mt, :], in0=EH[:, mt, hp, :],
                            scalar=exps[:, mt, hp:hp + 1],
                            in1=ctx_t[:, mt, :], op0=ALU.mult, op1=ALU.add)
            for mt in range(MT):
                if h == NH - 1:
                    # last head: both n-halves of this row-tile back-to-back,
                    # then its scores + softmax tail immediately — the tail
                    # for row-tile mt overlaps row-tile mt+1's matmuls, so
                    # almost nothing drains after the PE's last group.
                    emit_group(h, 0, mt, whns[0])
                    emit_group(h, 1, mt, whns[1])
                # scores[:, mt, h] = rowsum(EH_h * DH): single fused DVE op
                # (multiply with free-dim accumulation side-output)
                jt = junk([P, HID], BF16, "jsc")
                nc.vector.scalar_tensor_tensor(
                    out=jt[:, :], in0=EH[:, mt, h, :], scalar=1.0,
                    in1=st["DH"][:, mt, :], op0=ALU.bypass, op1=ALU.mult,
                    accum_out=scores[:, mt, h:h + 1])
                if h == NH - 1:
                    stats = st["stats"]
                    nc.scalar.activation(exps[:, mt, h:h + 1],
                                         scores[:, mt, h:h + 1], AF.Exp)
                    nc.vector.scalar_tensor_tensor(
                        out=ctx_t[:, mt, :], in0=EH[:, mt, h, :],
                        scalar=exps[:, mt, h:h + 1],
                        in1=ctx_t[:, mt, :], op0=ALU.mult, op1=ALU.add)
                    sumexp = stats[:, mt, 0:1]
                    rsum = stats[:, mt, 1:2]
                    nc.vector.tensor_reduce(sumexp, exps[:, mt, :],
                                            axis=AX.X, op=ALU.add)
                    nc.vector.reciprocal(rsum, sumexp)
                    nc.vector.tensor_scalar_mul(ctx_t[:, mt, :],
                                                ctx_t[:, mt, :], rsum)
        st.update(EH=EH, scores=scores, exps=exps, ctx_t=ctx_t)

    def p45(c):
        # context transpose to [feature, row] for fc1; the softmax 1/sumexp
        # normalization rides along for free as a diag matrix in place of
        # the transpose identity
        st = S[c]
        ctx_t = st["ctx_t"]
        ctxT = t8tile([P, KT, CH], "ctxT")
        for mt in range(MT):
            for g in range(2):
                trp = pstr("trc", dtype=BF16)
                for ft in range(4):
                    nc.tensor.transpose(
                        trp[:, ft * P:(ft + 1) * P],
                        ctx_t[:, mt, (g * 4 + ft) * P:(g * 4 + ft + 1) * P],
                        identity_bf)
                nc.vector.tensor_copy(
                    ctxT[:, g * 4:(g + 1) * 4, mt * P:(mt + 1) * P],
                    trp.rearrange("p (ft x) -> p ft x", ft=4))
        st["ctxT"] = ctxT

    def p6(c):
        st = S[c]
        x1T = t8tile([P, KT, CH], "x1T")
        for mh in range(2):
            w1 = wtile([P, KT, 512], "w1t")
            nc.sync.dma_start(
                w1, io["W1"][mh].rearrange("p (ko f) -> p ko f", ko=KT))
            for mi in range(4):
                m = mh * 4 + mi
                pm = psmm("pm_fc1")
                for kt in range(KT):
                    nc.tensor.matmul(pm, lhsT=w1[:, kt, mi * P:(mi + 1) * P],
                                     rhs=st["ctxT"][:, kt, :],
                                     start=(kt == 0), stop=(kt == KT - 1))
                nc.scalar.activation(x1T[:, m, :], pm, AF.Relu,
                                     bias=b1_pp[:, m:m + 1])
        st["x1T"] = x1T

    def p7(c):
        st = S[c]
        r0 = c * CH
        pq = psq("pq")
        for kt in range(KT):
            nc.tensor.matmul(pq, lhsT=W2sb[:, kt:kt + 1],
                             rhs=st["x1T"][:, kt, :],
                             start=(kt == 0), stop=(kt == KT - 1))
        q_rowT = acts.tile([1, CH], F32, tag="q_rowT", bufs=1, name="q_rowT")
        nc.scalar.activation(q_rowT[0:1, :], pq[0:1, :], AF.Identity,
                             bias=b2sb[0:1, 0:1])
        nc.sync.dma_start(q_ap[r0:r0 + CH, 0][None, :], q_rowT[0:1, :])

    STAGE_FNS = {"binit": lambda c: binit(), "p1": p1, "p1b": p1b,
                 "p2a": p2a, "p2b": p2b, "p3": p3, "p45": p45, "p6": p6,
                 "p7": p7}

    # Emission order: chunk 1's enc GEMMs are emitted inside chunk 0's
    # softmax tail so the PE stays busy while the DVE finishes the last
    # head / normalization.
    order = [(0, "binit"), (0, "p1"), (1, "p1"),
             (0, "p1b"), (0, "p2a"), (0, "p2b"), (0, "p3"),
             (1, "p1b"),
             (0, "p45"), (0, "p6"), (0, "p7"),
             (1, "p2a"), (1, "p2b"), (1, "p3"),
             (1, "p45"), (1, "p6"), (1, "p7")]

    for c, nm in order:
        sc_(c, nm)
        STAGE_FNS[nm](c)
    sc_(0, None)


_NC_CACHE = None


def build():
    global _NC_CACHE
    if _NC_CACHE is not None:
        return _NC_CACHE
    nc = bacc.Bacc(trn_type="TRN2", target_bir_lowering=False, debug=False,
                   enable_asserts=False)
    io = {}
    # all tensors are host-pre-arranged so every DMA is contiguous per
    # partition (8KB descriptors): W[.., p, ko*f] = W_orig[ko*128+p, f]
    shapes_bf = {
        "pk": [P, 11 * RPC],
        "W_eh": [2, P, KT * 512],
        "W_heads": [NH, 2, P, KT * 512],
        "W1": [2, P, KT * 512],
        "W2": [P, KT],
        "Wfa": [DEC_IN + 1, HID],
        "b_heads": [1, NH * HID],
    }
    shapes_f32 = {
        "bias_pp": [P, 3 * KT], "b2": [1],
    }
    for name, shp in shapes_bf.items():
        io[name] = nc.dram_tensor(name, shp, BF16, kind="ExternalInput").ap()
    for name, shp in shapes_f32.items():
        io[name] = nc.dram_tensor(name, shp, F32, kind="ExternalInput").ap()
    io["q"] = nc.dram_tensor("q", [RPC, 1], F32, kind="ExternalOutput").ap()

    from contextlib import ExitStack
    with tile.TileContext(nc) as tc, ExitStack() as ctx:
        _body(nc, tc, io, ctx)
    nc.compile()
    _NC_CACHE = nc
    return nc


def _ktile(w, nhalves):
    # [K, N] -> [nhalves, 128, KT*(N/nhalves)]: w_r[nh, p, ko*f] =
    # w[ko*128+p, nh*(N/nhalves)+f]
    K, N = w.shape
    nh = N // nhalves
    r = w.reshape(K // P, P, nhalves, nh).transpose(2, 1, 0, 3)
    return np.ascontiguousarray(r.reshape(nhalves, P, (K // P) * nh))


def _prep_inputs(inputs):
    import ml_dtypes
    bf16 = ml_dtypes.bfloat16
    arrs = {k: np.ascontiguousarray(np.asarray(v, dtype=np.float32))
            for k, v in inputs.items()}
    # host-side weight folding: dec_input feeds only decoder_H (no relu in
    # between), so W_fused = W_dec_in @ W_dh, b_fused = b_dec_in @ W_dh + b_dh
    wf = arrs["W_dec_in"] @ arrs["W_dh"]
    bfu = arrs["b_dec_in"] @ arrs["W_dh"] + arrs["b_dh"]
    wfa = np.ascontiguousarray(
        np.concatenate([wf, bfu[None, :]], axis=0)).astype(bf16)
    # s^T tiled as [128, 4*B]: sT[p, kt*B+r] = s[r, kt*128+p]
    sT = np.ascontiguousarray(
        arrs["s"].T.reshape(4, P, B).transpose(1, 0, 2)).astype(bf16)
    aT = np.ascontiguousarray(                # [129, B]: ones row appended
        np.concatenate([arrs["a"].T, np.ones((1, B), np.float32)],
                       axis=0)).astype(bf16)
    wcast = {
        "W_eh": _ktile(arrs["W_eh"], 2).astype(bf16),
        "W_heads": np.ascontiguousarray(np.stack(
            [_ktile(arrs["W_heads"][h], 2) for h in range(NH)])).astype(bf16),
        "W1": _ktile(arrs["W1"], 2).astype(bf16),
        "W2": np.ascontiguousarray(
            arrs["W2"].reshape(KT, P).T).astype(bf16),
        "Wfa": wfa,
    }
    wenc_t = _ktile(arrs["W_enc_in"][0:512], 1)[0].astype(bf16)  # [P, 4096]
    wencr = arrs["W_enc_in"][512:544].astype(bf16)               # [32, 1024]
    bcast = {
        "bias_pp": np.ascontiguousarray(np.concatenate(
            [arrs[k].reshape(KT, P).T for k in ("b_enc_in", "b_eh", "b1")],
            axis=1)),
        "b_heads": np.ascontiguousarray(
            arrs["b_heads"].reshape(1, NH * HID)).astype(bf16),
        "b2": arrs["b2"],
    }
    in_maps = []
    for c in range(NCORES):
        m = dict(bcast)
        m.update(wcast)
        # packed one-DMA startup tensor: [sT | wenc | wencr | a_own | a_oth+1]
        pkm = np.zeros((P, 11 * RPC), bf16)
        pkm[:, 0:4 * RPC] = sT[:, :, c * RPC:(c + 1) * RPC].reshape(P, 4 * RPC)
        pkm[:, 4 * RPC:8 * RPC] = wenc_t
        pkm[0:ENC_REM, 8 * RPC:8 * RPC + HID] = wencr
        aTc = aT[:, c * RPC:(c + 1) * RPC]
        pkm[0:ACTD, 9 * RPC:10 * RPC] = aTc[0:ACTD]
        pkm[0:DEC_IN + 1, 10 * RPC:11 * RPC] = aTc[ACTD:P + 1]
        m["pk"] = pkm
        in_maps.append(m)
    return in_maps


def run(inputs, trace=False):
    from concourse.bass_utils import run_bass_kernel_spmd
    nc = build()
    in_maps = _prep_inputs(inputs)
    res = run_bass_kernel_spmd(nc, in_maps, core_ids=list(range(NCORES)),
                               trace=trace)
    q = np.concatenate([r["q"] for r in res.results], axis=0)
    return np.ascontiguousarray(q.astype(np.float32)), res


def kernel(**inputs) -> np.ndarray:
    q, _ = run(inputs, trace=False)
    return q


# revision 54
# speedup vs baseline: 1.0217x; 1.0217x over previous
"""Trainium2 Bass kernel for nn_ATT_critic (attention critic network).

Strategy: data-parallel over batch across 8 NeuronCores (1024 rows/core),
2 chunks of 512 rows per core; all big GEMMs on the PE in bf16 (PSUM
accumulation in fp32).

Key design points:
  - host-side weight folding: W_fused = W_dec_in @ W_dh (+ fused bias) is a
    weight-only precompute, done once on the host and shipped augmented with
    its bias as a [97, HID] tensor (ones-row trick).
  - host-side transposes + bf16 cast: s^T and a^T are passed per-core in
    bf16 so the kernel DMAs activations directly in [feature, row] layout;
    all weights are pre-cast to bf16 (the PE's fp32r mode rounds to
    bf16-level precision anyway, so this costs ~nothing numerically and
    halves all weight DMA traffic, which was the p3 bottleneck).
  - bias via PSUM prefill: the heads-layer biases are partition-replicated
    once (GpSimd partition_broadcast) and copied into PSUM before each
    accumulation group (alternating DVE/ACT), replacing 128 K=1 ones-row
    bias matmuls (322 ns each on the PE).
  - p2b bias via K-augmentation (97-row stationary with a host-side ones
    row in aT).
  - emission order overlaps chunk 1's input DMA + enc GEMM with chunk 0's
    softmax/context tail; softmax normalization is interleaved with the
    context transposes per row-tile; the weight pool is quad-buffered so
    upcoming layers' W tiles prefetch during the heads GEMM.
"""

import numpy as np

import concourse.bass as bass
import concourse.tile as tile
from concourse import mybir
from concourse import bacc
from concourse.masks import make_identity

P = 128
B = 8192
NCORES = 8
RPC = B // NCORES        # rows per core
CH = 512                 # rows per chunk
NCHUNK = RPC // CH
MT = CH // P             # row tiles per chunk
HID = 1024
KT = HID // P            # k tiles over hidden dim
NH = 8                   # heads
ACTD = 32
DEC_IN = 96
ENC_REM = 32             # 544 - 512

F32 = mybir.dt.float32
BF16 = mybir.dt.bfloat16
AF = mybir.ActivationFunctionType
ALU = mybir.AluOpType
AX = mybir.AxisListType

F32_WEIGHTS = ["b_enc_in", "b_eh", "b_heads", "b1", "b2"]
BF16_WEIGHTS = ["W_enc_in", "W_eh", "W_heads", "W1", "W2"]


def _body(nc, tc, io, ctx):
    q_ap = io["q"]

    const = ctx.enter_context(tc.tile_pool(name="const", bufs=1))
    acts = ctx.enter_context(tc.tile_pool(name="acts", bufs=1))
    wp = ctx.enter_context(tc.tile_pool(name="wp", bufs=4))
    ps = ctx.enter_context(tc.tile_pool(name="ps", bufs=1, space="PSUM"))

    def wtile(shape, name):
        return wp.tile(shape, BF16, tag="w", bufs=4, name=name)

    def t8tile(shape, name, dtype=BF16):
        return acts.tile(shape, dtype, tag="t8", bufs=3, name=name)

    def junk(shape, dtype, name):
        return acts.tile(shape, dtype, tag="junk", bufs=2, name=name)

    def psmm(name, shape=None):
        return ps.tile(shape or [P, 512], F32, tag="mm", bufs=4, name=name)

    def pstr(name, dtype=F32):
        return ps.tile([P, 512], dtype, tag="tr", bufs=2, name=name)

    def psq(name):
        return ps.tile([1, 512], F32, tag="q", bufs=2, name=name)

    # ---------------- constants / one-time init ----------------
    identity_bf = const.tile([P, P], BF16, name="identity_bf")
    make_identity(nc, identity_bf)

    # [b_enc | b_eh | b1] merged into one DMA (fewer descriptors: descriptor
    # generation throughput is the startup bottleneck)
    bias_pp = const.tile([P, 3 * KT], F32, name="bias_pp")
    nc.sync.dma_start(bias_pp, io["bias_pp"])
    b_enc_pp = bias_pp[:, 0:KT]
    b_eh_pp = bias_pp[:, KT:2 * KT]
    b1_pp = bias_pp[:, 2 * KT:3 * KT]
    W2sb = const.tile([P, KT], BF16, name="W2sb")
    nc.sync.dma_start(W2sb, io["W2"])
    b2sb = const.tile([1, 1], F32, name="b2sb")
    nc.sync.dma_start(b2sb, io["b2"][None, :])
    # fused decoder weights (host-folded), bias as row 96
    wfa_sb = const.tile([DEC_IN + 1, HID], BF16, name="wfa_sb")
    nc.sync.dma_start(wfa_sb, io["Wfa"])

    b_full = const.tile([P, NH, HID], BF16, name="b_full")

    def binit():
        # head biases (host-cast bf16, one DMA to partition 0) replicated
        # across all partitions via GpSimd partition_broadcast
        bh_row = const.tile([1, NH * HID], BF16, name="bh_row")
        nc.sync.dma_start(bh_row, io["b_heads"])
        for h in range(NH):
            nc.gpsimd.partition_broadcast(
                b_full[:, h, :], bh_row[0:1, h * HID:(h + 1) * HID])

    # ---------------- per-chunk stages ----------------
    _scope_stack = [None]

    def sc_(c, nm):
        prev = _scope_stack.pop()
        if prev is not None:
            nc.leave_named_scope(prev[0], prev[1], False)
        if nm is not None:
            full = f"c{c}_{nm}"
            sid, _ = nc.enter_named_scope(full, False)
            _scope_stack.append((full, sid))
        else:
            _scope_stack.append(None)

    S = [dict() for _ in range(NCHUNK)]

    # all startup-critical tensors (W_enc tiled + remainder, s^T tiled, a^T
    # with its ones row) ride in host-packed DMAs: descriptor generation
    # throughput, not bandwidth, limits the startup, so 128 fat descriptors
    # beat ~450 small ones. Two pieces: chunk 0's (+ weights) is critical,
    # chunk 1's arrives while chunk 0 computes.
    PKW = 11264
    PK_CRIT = 8192
    pk = const.tile([P, PKW], BF16, name="pk")
    nc.sync.dma_start(pk[:, 0:PK_CRIT], io["pk"][:, 0:PK_CRIT])
    nc.sync.dma_start(pk[:, PK_CRIT:PKW], io["pk"][:, PK_CRIT:PKW])
    full = dict(
        wenc=pk[:, 0:4096].rearrange("p (ko f) -> p ko f", ko=4),
        wencr=pk[0:ENC_REM, 6144:7168],
    )
    _pkc = [dict(sT=pk[:, 4096:6144].rearrange("p (kt r) -> p kt r", kt=4),
                 aownT=pk[0:ACTD, 7168:7680],
                 aothT=pk[0:DEC_IN + 1, 7680:8192]),
            dict(sT=pk[:, 8192:10240].rearrange("p (kt r) -> p kt r", kt=4),
                 aownT=pk[0:ACTD, 10240:10752],
                 aothT=pk[0:DEC_IN + 1, 10752:11264])]

    def p1(c):
        S[c].update(_pkc[c])

    def p1b(c):
        st = S[c]
        wenc = full["wenc"]
        enc_inT = t8tile([P, KT, CH], "enc_inT")
        for m in range(KT):
            pm = psmm("pm_enc")
            for kt in range(4):
                nc.tensor.matmul(pm, lhsT=wenc[:, kt, m * P:(m + 1) * P],
                                 rhs=st["sT"][:, kt, :], start=(kt == 0),
                                 stop=False)
            nc.tensor.matmul(pm, lhsT=full["wencr"][:, m * P:(m + 1) * P],
                             rhs=st["aownT"], start=False, stop=True)
            nc.scalar.activation(enc_inT[:, m, :], pm, AF.Identity,
                                 bias=b_enc_pp[:, m:m + 1])
        st["enc_inT"] = enc_inT

    def p2a(c):
        st = S[c]
        encHT = t8tile([P, KT, CH], "encHT")
        for mh in range(2):
            weh = wtile([P, KT, 512], "weh")
            nc.sync.dma_start(
                weh, io["W_eh"][mh].rearrange("p (ko f) -> p ko f", ko=KT))
            for mi in range(4):
                m = mh * 4 + mi
                pm = psmm("pm_eh")
                for kt in range(KT):
                    nc.tensor.matmul(pm, lhsT=weh[:, kt, mi * P:(mi + 1) * P],
                                     rhs=st["enc_inT"][:, kt, :],
                                     start=(kt == 0), stop=(kt == KT - 1))
                nc.scalar.activation(encHT[:, m, :], pm, AF.Relu,
                                     bias=b_eh_pp[:, m:m + 1])
        st["encHT"] = encHT

    def p2b(c):
        # DH = relu([a_others, 1] @ [W_fused; b_fused]) : K=97, no bias matmul
        st = S[c]
        DH = acts.tile([P, MT, HID], BF16, tag="dh", bufs=1, name="DH")
        for mt in range(MT):
            for n in range(2):
                pm = psmm("pm_dh")
                nc.tensor.matmul(pm,
                                 lhsT=st["aothT"][:, mt * P:(mt + 1) * P],
                                 rhs=wfa_sb[:, n * 512:(n + 1) * 512],
                                 start=True, stop=True)
                nc.scalar.activation(DH[:, mt, n * 512:(n + 1) * 512], pm,
                                     AF.Relu)
        st["DH"] = DH

    def p3(c):
        st = S[c]
        EH = acts.tile([P, MT, NH, HID], BF16, tag="eh", bufs=1, name="EH")
        scores = acts.tile([P, MT, NH], F32, tag="scores", bufs=2, name="scores")

        def emit_group(h, n, mt, whn):
            pm = psmm("pm_hd")
            # bias prefill from the replicated tile (alternating DVE/ACT,
            # off the PE critical path); matmuls accumulate on top.
            # GpSimd cannot write PSUM on TRN2.
            if mt == 0:
                nc.vector.tensor_copy(pm, b_full[:, h, n * 512:(n + 1) * 512])
            else:
                nc.scalar.activation(pm, b_full[:, h, n * 512:(n + 1) * 512],
                                     AF.Copy)
            for kt in range(KT):
                nc.tensor.matmul(
                    pm, lhsT=st["encHT"][:, kt, mt * P:(mt + 1) * P],
                    rhs=whn[:, kt, :], start=False,
                    stop=(kt == KT - 1), skip_group_check=True)
            nc.scalar.activation(EH[:, mt, h, n * 512:(n + 1) * 512],
                                 pm, AF.Relu)

        for h in range(NH):
            whns = []
            for n in range(2):
                whn = wtile([P, KT, 512], f"whn{h}_{n}")
                nc.sync.dma_start(
                    whn, io["W_heads"][h, n].rearrange("p (ko f) -> p ko f",
                                                       ko=KT))
                whns.append(whn)
                if h < NH - 1:
                    for mt in range(MT):
                        emit_group(h, n, mt, whn)
            def ctx_accum(hp, mt):
                # head hp's scores are complete: fold its (unnormalized)
                # softmax contribution into the context, spread evenly so
                # the DVE queue never floods and delays the PSUM bias
                # prefills. exp without max-subtraction is safe: scores ~< 12.
                nc.scalar.activation(exps[:, mt, hp:hp + 1],
                                     scores[:, mt, hp:hp + 1], AF.Exp)
                if hp == 0:
                    nc.vector.tensor_scalar_mul(ctx_t[:, mt, :],
                                                EH[:, mt, 0, :],
                                                exps[:, mt, 0:1])
                else:
                    nc.vector.scalar_tensor_tensor(
                        out=ctx_t[:, mt, :], in0=EH[:, mt, hp, :],
                        scalar=exps[:, mt, hp:hp + 1],
                        in1=ctx_t[:, mt, :], op0=ALU.mult, op1=ALU.add)

            if h == 1:
                exps = acts.tile([P, MT, NH], F32, tag="attn", bufs=2,
                                 name="exps")
                ctx_t = t8tile([P, MT, HID], "ctx_t")
                st["stats"] = acts.tile([P, MT, 2], F32, tag="stats",
                                        bufs=2, name="stats")
            if 1 <= h < NH - 1:
                for mt in range(MT):
                    ctx_accum(h - 1, mt)
            for mt in range(MT):
                if h == NH - 1:
                    # previous head's context fold-in rides per row-tile here
                    # so the DVE burst never queues ahead of this head's
                    # PSUM prefills
                    ctx_accum(h - 1, mt)
                    # last head: both n-halves of this row-tile back-to-back,
                    # then its scores + softmax tail immediately — the tail
                    # for row-tile mt overlaps row-tile mt+1's matmuls, so
                    # almost nothing drains after the PE's last group.
                    emit_group(h, 0, mt, whns[0])
                    emit_group(h, 1, mt, whns[1])
                # scores[:, mt, h] = rowsum(EH_h * DH): single fused DVE op
                # (multiply with free-dim accumulation side-output)
                jt = junk([P, HID], BF16, "jsc")
                nc.vector.scalar_tensor_tensor(
                    out=jt[:, :], in0=EH[:, mt, h, :], scalar=1.0,
                    in1=st["DH"][:, mt, :], op0=ALU.bypass, op1=ALU.mult,
                    accum_out=scores[:, mt, h:h + 1])
                if h == NH - 1:
                    stats = st["stats"]
                    nc.scalar.activation(exps[:, mt, h:h + 1],
                                         scores[:, mt, h:h + 1], AF.Exp)
                    nc.vector.scalar_tensor_tensor(
                        out=ctx_t[:, mt, :], in0=EH[:, mt, h, :],
                        scalar=exps[:, mt, h:h + 1],
                        in1=ctx_t[:, mt, :], op0=ALU.mult, op1=ALU.add)
                    sumexp = stats[:, mt, 0:1]
                    rsum = stats[:, mt, 1:2]
                    nc.vector.tensor_reduce(sumexp, exps[:, mt, :],
                                            axis=AX.X, op=ALU.add)
                    nc.vector.reciprocal(rsum, sumexp)
                    nc.vector.tensor_scalar_mul(ctx_t[:, mt, :],
                                                ctx_t[:, mt, :], rsum)
        st.update(EH=EH, scores=scores, exps=exps, ctx_t=ctx_t)

    def p45(c):
        # context transpose to [feature, row] for fc1; the softmax 1/sumexp
        # normalization rides along for free as a diag matrix in place of
        # the transpose identity
        st = S[c]
        ctx_t = st["ctx_t"]
        ctxT = t8tile([P, KT, CH], "ctxT")
        for mt in range(MT):
            for g in range(2):
                trp = pstr("trc", dtype=BF16)
                for ft in range(4):
                    nc.tensor.transpose(
                        trp[:, ft * P:(ft + 1) * P],
                        ctx_t[:, mt, (g * 4 + ft) * P:(g * 4 + ft + 1) * P],
                        identity_bf)
                nc.vector.tensor_copy(
                    ctxT[:, g * 4:(g + 1) * 4, mt * P:(mt + 1) * P],
                    trp.rearrange("p (ft x) -> p ft x", ft=4))
        st["ctxT"] = ctxT

    def p6(c):
        st = S[c]
        x1T = t8tile([P, KT, CH], "x1T")
        for mh in range(2):
            w1 = wtile([P, KT, 512], "w1t")
            nc.sync.dma_start(
                w1, io["W1"][mh].rearrange("p (ko f) -> p ko f", ko=KT))
            for mi in range(4):
                m = mh * 4 + mi
                pm = psmm("pm_fc1")
                for kt in range(KT):
                    nc.tensor.matmul(pm, lhsT=w1[:, kt, mi * P:(mi + 1) * P],
                                     rhs=st["ctxT"][:, kt, :],
                                     start=(kt == 0), stop=(kt == KT - 1))
                nc.scalar.activation(x1T[:, m, :], pm, AF.Relu,
                                     bias=b1_pp[:, m:m + 1])
        st["x1T"] = x1T

    def p7(c):
        st = S[c]
        r0 = c * CH
        pq = psq("pq")
        for kt in range(KT):
            nc.tensor.matmul(pq, lhsT=W2sb[:, kt:kt + 1],
                             rhs=st["x1T"][:, kt, :],
                             start=(kt == 0), stop=(kt == KT - 1))
        q_rowT = acts.tile([1, CH], F32, tag="q_rowT", bufs=1, name="q_rowT")
        nc.scalar.activation(q_rowT[0:1, :], pq[0:1, :], AF.Identity,
                             bias=b2sb[0:1, 0:1])
        nc.sync.dma_start(q_ap[r0:r0 + CH, 0][None, :], q_rowT[0:1, :])

    STAGE_FNS = {"binit": lambda c: binit(), "p1": p1, "p1b": p1b,
                 "p2a": p2a, "p2b": p2b, "p3": p3, "p45": p45, "p6": p6,
                 "p7": p7}

    # Emission order: chunk 1's enc GEMMs are emitted inside chunk 0's
    # softmax tail so the PE stays busy while the DVE finishes the last
    # head / normalization.
    order = [(0, "binit"), (0, "p1"), (1, "p1"),
             (0, "p1b"), (0, "p2a"), (0, "p2b"), (0, "p3"),
             (1, "p1b"),
             (0, "p45"), (0, "p6"), (0, "p7"),
             (1, "p2a"), (1, "p2b"), (1, "p3"),
             (1, "p45"), (1, "p6"), (1, "p7")]

    for c, nm in order:
        sc_(c, nm)
        STAGE_FNS[nm](c)
    sc_(0, None)


_NC_CACHE = None


def build():
    global _NC_CACHE
    if _NC_CACHE is not None:
        return _NC_CACHE
    nc = bacc.Bacc(trn_type="TRN2", target_bir_lowering=False, debug=False,
                   enable_asserts=False)
    io = {}
    # all tensors are host-pre-arranged so every DMA is contiguous per
    # partition (8KB descriptors): W[.., p, ko*f] = W_orig[ko*128+p, f]
    shapes_bf = {
        "pk": [P, 11 * RPC],
        "W_eh": [2, P, KT * 512],
        "W_heads": [NH, 2, P, KT * 512],
        "W1": [2, P, KT * 512],
        "W2": [P, KT],
        "Wfa": [DEC_IN + 1, HID],
        "b_heads": [1, NH * HID],
    }
    shapes_f32 = {
        "bias_pp": [P, 3 * KT], "b2": [1],
    }
    for name, shp in shapes_bf.items():
        io[name] = nc.dram_tensor(name, shp, BF16, kind="ExternalInput").ap()
    for name, shp in shapes_f32.items():
        io[name] = nc.dram_tensor(name, shp, F32, kind="ExternalInput").ap()
    io["q"] = nc.dram_tensor("q", [RPC, 1], F32, kind="ExternalOutput").ap()

    from contextlib import ExitStack
    with tile.TileContext(nc) as tc, ExitStack() as ctx:
        _body(nc, tc, io, ctx)
    nc.compile()
    _NC_CACHE = nc
    return nc


def _ktile(w, nhalves):
    # [K, N] -> [nhalves, 128, KT*(N/nhalves)]: w_r[nh, p, ko*f] =
    # w[ko*128+p, nh*(N/nhalves)+f]
    K, N = w.shape
    nh = N // nhalves
    r = w.reshape(K // P, P, nhalves, nh).transpose(2, 1, 0, 3)
    return np.ascontiguousarray(r.reshape(nhalves, P, (K // P) * nh))


def _prep_inputs(inputs):
    import ml_dtypes
    bf16 = ml_dtypes.bfloat16
    arrs = {k: np.ascontiguousarray(np.asarray(v, dtype=np.float32))
            for k, v in inputs.items()}
    # host-side weight folding: dec_input feeds only decoder_H (no relu in
    # between), so W_fused = W_dec_in @ W_dh, b_fused = b_dec_in @ W_dh + b_dh
    wf = arrs["W_dec_in"] @ arrs["W_dh"]
    bfu = arrs["b_dec_in"] @ arrs["W_dh"] + arrs["b_dh"]
    wfa = np.ascontiguousarray(
        np.concatenate([wf, bfu[None, :]], axis=0)).astype(bf16)
    # s^T tiled as [128, 4*B]: sT[p, kt*B+r] = s[r, kt*128+p]
    sT = np.ascontiguousarray(
        arrs["s"].T.reshape(4, P, B).transpose(1, 0, 2)).astype(bf16)
    aT = np.ascontiguousarray(                # [129, B]: ones row appended
        np.concatenate([arrs["a"].T, np.ones((1, B), np.float32)],
                       axis=0)).astype(bf16)
    wcast = {
        "W_eh": _ktile(arrs["W_eh"], 2).astype(bf16),
        "W_heads": np.ascontiguousarray(np.stack(
            [_ktile(arrs["W_heads"][h], 2) for h in range(NH)])).astype(bf16),
        "W1": _ktile(arrs["W1"], 2).astype(bf16),
        "W2": np.ascontiguousarray(
            arrs["W2"].reshape(KT, P).T).astype(bf16),
        "Wfa": wfa,
    }
    wenc_t = _ktile(arrs["W_enc_in"][0:512], 1)[0].astype(bf16)  # [P, 4096]
    wencr = arrs["W_enc_in"][512:544].astype(bf16)               # [32, 1024]
    bcast = {
        "bias_pp": np.ascontiguousarray(np.concatenate(
            [arrs[k].reshape(KT, P).T for k in ("b_enc_in", "b_eh", "b1")],
            axis=1)),
        "b_heads": np.ascontiguousarray(
            arrs["b_heads"].reshape(1, NH * HID)).astype(bf16),
        "b2": arrs["b2"],
    }
    in_maps = []
    for c in range(NCORES):
        m = dict(bcast)
        m.update(wcast)
        # packed startup tensor: [wenc | sT_c0 | wencr | aown_c0 | aoth_c0 |
        # sT_c1 | aown_c1 | aoth_c1]
        pkm = np.zeros((P, 11264), bf16)
        pkm[:, 0:4096] = wenc_t
        pkm[0:ENC_REM, 6144:7168] = wencr
        for ch in range(2):
            r0 = c * RPC + ch * CH
            o = 4096 if ch == 0 else 8192
            oa = 7168 if ch == 0 else 10240
            pkm[:, o:o + 2048] = sT[:, :, r0:r0 + CH].reshape(P, 4 * CH)
            aTc = aT[:, r0:r0 + CH]
            pkm[0:ACTD, oa:oa + CH] = aTc[0:ACTD]
            pkm[0:DEC_IN + 1, oa + CH:oa + 2 * CH] = aTc[ACTD:P + 1]
        m["pk"] = pkm
        in_maps.append(m)
    return in_maps


def run(inputs, trace=False):
    from concourse.bass_utils import run_bass_kernel_spmd
    nc = build()
    in_maps = _prep_inputs(inputs)
    res = run_bass_kernel_spmd(nc, in_maps, core_ids=list(range(NCORES)),
                               trace=trace)
    q = np.concatenate([r["q"] for r in res.results], axis=0)
    return np.ascontiguousarray(q.astype(np.float32)), res


def kernel(**inputs) -> np.ndarray:
    q, _ = run(inputs, trace=False)
    return q


# revision 57
# speedup vs baseline: 1.0402x; 1.0182x over previous
"""Trainium2 Bass kernel for nn_ATT_critic (attention critic network).

Strategy: data-parallel over batch across 8 NeuronCores (1024 rows/core),
2 chunks of 512 rows per core; all big GEMMs on the PE in bf16 (PSUM
accumulation in fp32).

Key design points:
  - host-side weight folding: W_fused = W_dec_in @ W_dh (+ fused bias) is a
    weight-only precompute, done once on the host and shipped augmented with
    its bias as a [97, HID] tensor (ones-row trick).
  - host-side transposes + bf16 cast: s^T and a^T are passed per-core in
    bf16 so the kernel DMAs activations directly in [feature, row] layout;
    all weights are pre-cast to bf16 (the PE's fp32r mode rounds to
    bf16-level precision anyway, so this costs ~nothing numerically and
    halves all weight DMA traffic, which was the p3 bottleneck).
  - bias via PSUM prefill: the heads-layer biases are partition-replicated
    once (GpSimd partition_broadcast) and copied into PSUM before each
    accumulation group (alternating DVE/ACT), replacing 128 K=1 ones-row
    bias matmuls (322 ns each on the PE).
  - p2b bias via K-augmentation (97-row stationary with a host-side ones
    row in aT).
  - emission order overlaps chunk 1's input DMA + enc GEMM with chunk 0's
    softmax/context tail; softmax normalization is interleaved with the
    context transposes per row-tile; the weight pool is quad-buffered so
    upcoming layers' W tiles prefetch during the heads GEMM.
"""

import numpy as np

import concourse.bass as bass
import concourse.tile as tile
from concourse import mybir
from concourse import bacc
from concourse.masks import make_identity

P = 128
B = 8192
NCORES = 8
RPC = B // NCORES        # rows per core
CH = 512                 # rows per chunk
NCHUNK = RPC // CH
MT = CH // P             # row tiles per chunk
HID = 1024
KT = HID // P            # k tiles over hidden dim
NH = 8                   # heads
ACTD = 32
DEC_IN = 96
ENC_REM = 32             # 544 - 512

F32 = mybir.dt.float32
BF16 = mybir.dt.bfloat16
AF = mybir.ActivationFunctionType
ALU = mybir.AluOpType
AX = mybir.AxisListType

F32_WEIGHTS = ["b_enc_in", "b_eh", "b_heads", "b1", "b2"]
BF16_WEIGHTS = ["W_enc_in", "W_eh", "W_heads", "W1", "W2"]


def _body(nc, tc, io, ctx):
    q_ap = io["q"]

    const = ctx.enter_context(tc.tile_pool(name="const", bufs=1))
    acts = ctx.enter_context(tc.tile_pool(name="acts", bufs=1))
    wp = ctx.enter_context(tc.tile_pool(name="wp", bufs=5))
    ps = ctx.enter_context(tc.tile_pool(name="ps", bufs=1, space="PSUM"))

    def wtile(shape, name):
        return wp.tile(shape, BF16, tag="w", bufs=5, name=name)

    def t8tile(shape, name, dtype=BF16):
        return acts.tile(shape, dtype, tag="t8", bufs=3, name=name)

    def junk(shape, dtype, name):
        return acts.tile(shape, dtype, tag="junk", bufs=2, name=name)

    def psmm(name, shape=None):
        return ps.tile(shape or [P, 512], F32, tag="mm", bufs=4, name=name)

    def pstr(name, dtype=F32):
        return ps.tile([P, 512], dtype, tag="tr", bufs=2, name=name)

    def psq(name):
        return ps.tile([1, 512], F32, tag="q", bufs=2, name=name)

    # ---------------- constants / one-time init ----------------
    identity_bf = const.tile([P, P], BF16, name="identity_bf")
    make_identity(nc, identity_bf)

    # [b_enc | b_eh | b1] merged into one DMA (fewer descriptors: descriptor
    # generation throughput is the startup bottleneck)
    bias_pp = const.tile([P, 3 * KT], F32, name="bias_pp")
    nc.sync.dma_start(bias_pp, io["bias_pp"])
    b_enc_pp = bias_pp[:, 0:KT]
    b_eh_pp = bias_pp[:, KT:2 * KT]
    b1_pp = bias_pp[:, 2 * KT:3 * KT]
    W2sb = const.tile([P, KT], BF16, name="W2sb")
    nc.sync.dma_start(W2sb, io["W2"])
    b2sb = const.tile([1, 1], F32, name="b2sb")
    nc.sync.dma_start(b2sb, io["b2"][None, :])
    # fused decoder weights (host-folded), bias as row 96
    wfa_sb = const.tile([DEC_IN + 1, HID], BF16, name="wfa_sb")
    nc.sync.dma_start(wfa_sb, io["Wfa"])

    b_full = const.tile([P, NH, HID], BF16, name="b_full")

    def binit():
        # head biases (host-cast bf16, one DMA to partition 0) replicated
        # across all partitions via GpSimd partition_broadcast
        bh_row = const.tile([1, NH * HID], BF16, name="bh_row")
        nc.sync.dma_start(bh_row, io["b_heads"])
        for h in range(NH):
            nc.gpsimd.partition_broadcast(
                b_full[:, h, :], bh_row[0:1, h * HID:(h + 1) * HID])

    # ---------------- per-chunk stages ----------------
    _scope_stack = [None]

    def sc_(c, nm):
        prev = _scope_stack.pop()
        if prev is not None:
            nc.leave_named_scope(prev[0], prev[1], False)
        if nm is not None:
            full = f"c{c}_{nm}"
            sid, _ = nc.enter_named_scope(full, False)
            _scope_stack.append((full, sid))
        else:
            _scope_stack.append(None)

    S = [dict() for _ in range(NCHUNK)]

    # all startup-critical tensors (W_enc tiled + remainder, s^T tiled, a^T
    # with its ones row) ride in host-packed DMAs: descriptor generation
    # throughput, not bandwidth, limits the startup, so 128 fat descriptors
    # beat ~450 small ones. Two pieces: chunk 0's (+ weights) is critical,
    # chunk 1's arrives while chunk 0 computes.
    PKW = 11264
    PK_CRIT = 8192
    pk = const.tile([P, PKW], BF16, name="pk")
    nc.sync.dma_start(pk[:, 0:PK_CRIT], io["pk"][:, 0:PK_CRIT])
    nc.sync.dma_start(pk[:, PK_CRIT:PKW], io["pk"][:, PK_CRIT:PKW])
    full = dict(
        wenc=pk[:, 0:4096].rearrange("p (ko f) -> p ko f", ko=4),
        wencr=pk[0:ENC_REM, 6144:7168],
    )
    _pkc = [dict(sT=pk[:, 4096:6144].rearrange("p (kt r) -> p kt r", kt=4),
                 aownT=pk[0:ACTD, 7168:7680],
                 aothT=pk[0:DEC_IN + 1, 7680:8192]),
            dict(sT=pk[:, 8192:10240].rearrange("p (kt r) -> p kt r", kt=4),
                 aownT=pk[0:ACTD, 10240:10752],
                 aothT=pk[0:DEC_IN + 1, 10752:11264])]

    def p1(c):
        S[c].update(_pkc[c])

    def p1b(c):
        st = S[c]
        wenc = full["wenc"]
        enc_inT = t8tile([P, KT, CH], "enc_inT")
        for m in range(KT):
            pm = psmm("pm_enc")
            for kt in range(4):
                nc.tensor.matmul(pm, lhsT=wenc[:, kt, m * P:(m + 1) * P],
                                 rhs=st["sT"][:, kt, :], start=(kt == 0),
                                 stop=False)
            nc.tensor.matmul(pm, lhsT=full["wencr"][:, m * P:(m + 1) * P],
                             rhs=st["aownT"], start=False, stop=True)
            nc.scalar.activation(enc_inT[:, m, :], pm, AF.Identity,
                                 bias=b_enc_pp[:, m:m + 1])
        st["enc_inT"] = enc_inT

    def p2a(c):
        st = S[c]
        encHT = t8tile([P, KT, CH], "encHT")
        for mh in range(2):
            weh = wtile([P, KT, 512], "weh")
            nc.sync.dma_start(
                weh, io["W_eh"][mh].rearrange("p (ko f) -> p ko f", ko=KT))
            for mi in range(4):
                m = mh * 4 + mi
                pm = psmm("pm_eh")
                for kt in range(KT):
                    nc.tensor.matmul(pm, lhsT=weh[:, kt, mi * P:(mi + 1) * P],
                                     rhs=st["enc_inT"][:, kt, :],
                                     start=(kt == 0), stop=(kt == KT - 1))
                nc.scalar.activation(encHT[:, m, :], pm, AF.Relu,
                                     bias=b_eh_pp[:, m:m + 1])
        st["encHT"] = encHT

    def p2b(c):
        # DH = relu([a_others, 1] @ [W_fused; b_fused]) : K=97, no bias matmul
        st = S[c]
        DH = acts.tile([P, MT, HID], BF16, tag="dh", bufs=1, name="DH")
        for mt in range(MT):
            for n in range(2):
                pm = psmm("pm_dh")
                nc.tensor.matmul(pm,
                                 lhsT=st["aothT"][:, mt * P:(mt + 1) * P],
                                 rhs=wfa_sb[:, n * 512:(n + 1) * 512],
                                 start=True, stop=True)
                nc.scalar.activation(DH[:, mt, n * 512:(n + 1) * 512], pm,
                                     AF.Relu)
        st["DH"] = DH

    def p3(c):
        st = S[c]
        EH = acts.tile([P, MT, NH, HID], BF16, tag="eh", bufs=1, name="EH")
        scores = acts.tile([P, MT, NH], F32, tag="scores", bufs=2, name="scores")

        def emit_group(h, n, mt, whn):
            pm = psmm("pm_hd")
            # bias prefill from the replicated tile (alternating DVE/ACT,
            # off the PE critical path); matmuls accumulate on top.
            # GpSimd cannot write PSUM on TRN2.
            if mt == 0:
                nc.vector.tensor_copy(pm, b_full[:, h, n * 512:(n + 1) * 512])
            else:
                nc.scalar.activation(pm, b_full[:, h, n * 512:(n + 1) * 512],
                                     AF.Copy)
            for kt in range(KT):
                nc.tensor.matmul(
                    pm, lhsT=st["encHT"][:, kt, mt * P:(mt + 1) * P],
                    rhs=whn[:, kt, :], start=False,
                    stop=(kt == KT - 1), skip_group_check=True)
            nc.scalar.activation(EH[:, mt, h, n * 512:(n + 1) * 512],
                                 pm, AF.Relu)

        for h in range(NH):
            whns = []
            for n in range(2):
                whn = wtile([P, KT, 512], f"whn{h}_{n}")
                nc.sync.dma_start(
                    whn, io["W_heads"][h, n].rearrange("p (ko f) -> p ko f",
                                                       ko=KT))
                whns.append(whn)
                if h < NH - 1:
                    for mt in range(MT):
                        emit_group(h, n, mt, whn)
            def ctx_accum(hp, mt):
                # head hp's scores are complete: fold its (unnormalized)
                # softmax contribution into the context, spread evenly so
                # the DVE queue never floods and delays the PSUM bias
                # prefills. exp without max-subtraction is safe: scores ~< 12.
                nc.scalar.activation(exps[:, mt, hp:hp + 1],
                                     scores[:, mt, hp:hp + 1], AF.Exp)
                if hp == 0:
                    nc.vector.tensor_scalar_mul(ctx_t[:, mt, :],
                                                EH[:, mt, 0, :],
                                                exps[:, mt, 0:1])
                else:
                    nc.vector.scalar_tensor_tensor(
                        out=ctx_t[:, mt, :], in0=EH[:, mt, hp, :],
                        scalar=exps[:, mt, hp:hp + 1],
                        in1=ctx_t[:, mt, :], op0=ALU.mult, op1=ALU.add)

            if h == 1:
                exps = acts.tile([P, MT, NH], F32, tag="attn", bufs=2,
                                 name="exps")
                ctx_t = t8tile([P, MT, HID], "ctx_t")
                st["stats"] = acts.tile([P, MT, 2], F32, tag="stats",
                                        bufs=2, name="stats")
            if 1 <= h < NH - 1:
                for mt in range(MT):
                    ctx_accum(h - 1, mt)
            for mt in range(MT):
                if h == NH - 1:
                    # previous head's context fold-in rides per row-tile here
                    # so the DVE burst never queues ahead of this head's
                    # PSUM prefills
                    ctx_accum(h - 1, mt)
                    # last head: both n-halves of this row-tile back-to-back,
                    # then its scores + softmax tail immediately — the tail
                    # for row-tile mt overlaps row-tile mt+1's matmuls, so
                    # almost nothing drains after the PE's last group.
                    emit_group(h, 0, mt, whns[0])
                    emit_group(h, 1, mt, whns[1])
                # scores[:, mt, h] = rowsum(EH_h * DH): single fused DVE op
                # (multiply with free-dim accumulation side-output)
                jt = junk([P, HID], BF16, "jsc")
                nc.vector.scalar_tensor_tensor(
                    out=jt[:, :], in0=EH[:, mt, h, :], scalar=1.0,
                    in1=st["DH"][:, mt, :], op0=ALU.bypass, op1=ALU.mult,
                    accum_out=scores[:, mt, h:h + 1])
                if h == NH - 1:
                    stats = st["stats"]
                    nc.scalar.activation(exps[:, mt, h:h + 1],
                                         scores[:, mt, h:h + 1], AF.Exp)
                    nc.vector.scalar_tensor_tensor(
                        out=ctx_t[:, mt, :], in0=EH[:, mt, h, :],
                        scalar=exps[:, mt, h:h + 1],
                        in1=ctx_t[:, mt, :], op0=ALU.mult, op1=ALU.add)
                    sumexp = stats[:, mt, 0:1]
                    rsum = stats[:, mt, 1:2]
                    nc.vector.tensor_reduce(sumexp, exps[:, mt, :],
                                            axis=AX.X, op=ALU.add)
                    nc.vector.reciprocal(rsum, sumexp)
                    nc.vector.tensor_scalar_mul(ctx_t[:, mt, :],
                                                ctx_t[:, mt, :], rsum)
        st.update(EH=EH, scores=scores, exps=exps, ctx_t=ctx_t)

    def p45(c):
        # context transpose to [feature, row] for fc1; the softmax 1/sumexp
        # normalization rides along for free as a diag matrix in place of
        # the transpose identity
        st = S[c]
        ctx_t = st["ctx_t"]
        ctxT = t8tile([P, KT, CH], "ctxT")
        for mt in range(MT):
            for g in range(2):
                trp = pstr("trc", dtype=BF16)
                for ft in range(4):
                    nc.tensor.transpose(
                        trp[:, ft * P:(ft + 1) * P],
                        ctx_t[:, mt, (g * 4 + ft) * P:(g * 4 + ft + 1) * P],
                        identity_bf)
                nc.vector.tensor_copy(
                    ctxT[:, g * 4:(g + 1) * 4, mt * P:(mt + 1) * P],
                    trp.rearrange("p (ft x) -> p ft x", ft=4))
        st["ctxT"] = ctxT

    def p6(c):
        st = S[c]
        x1T = t8tile([P, KT, CH], "x1T")
        pq = psq("pq")
        for mh in range(2):
            w1 = wtile([P, KT, 512], "w1t")
            nc.sync.dma_start(
                w1, io["W1"][mh].rearrange("p (ko f) -> p ko f", ko=KT))
            for mi in range(4):
                m = mh * 4 + mi
                pm = psmm("pm_fc1")
                for kt in range(KT):
                    nc.tensor.matmul(pm, lhsT=w1[:, kt, mi * P:(mi + 1) * P],
                                     rhs=st["ctxT"][:, kt, :],
                                     start=(kt == 0), stop=(kt == KT - 1))
                nc.scalar.activation(x1T[:, m, :], pm, AF.Relu,
                                     bias=b1_pp[:, m:m + 1])
            # fc2 rides behind each half's evictions so the final output
            # eviction starts right after the last fc1 matmul
            for kt in range(mh * 4, mh * 4 + 4):
                nc.tensor.matmul(pq, lhsT=W2sb[:, kt:kt + 1],
                                 rhs=x1T[:, kt, :],
                                 start=(kt == 0), stop=(kt == KT - 1))
        st["pq"] = pq

    def p7(c):
        st = S[c]
        r0 = c * CH
        q_rowT = acts.tile([1, CH], F32, tag="q_rowT", bufs=1, name="q_rowT")
        nc.scalar.activation(q_rowT[0:1, :], st["pq"][0:1, :], AF.Identity,
                             bias=b2sb[0:1, 0:1])
        nc.sync.dma_start(q_ap[r0:r0 + CH, 0][None, :], q_rowT[0:1, :])

    STAGE_FNS = {"binit": lambda c: binit(), "p1": p1, "p1b": p1b,
                 "p2a": p2a, "p2b": p2b, "p3": p3, "p45": p45, "p6": p6,
                 "p7": p7}

    # Emission order: chunk 1's enc GEMMs are emitted inside chunk 0's
    # softmax tail so the PE stays busy while the DVE finishes the last
    # head / normalization.
    order = [(0, "binit"), (0, "p1"), (1, "p1"),
             (0, "p1b"), (0, "p2a"), (0, "p2b"), (0, "p3"),
             (1, "p1b"),
             (0, "p45"), (0, "p6"), (0, "p7"),
             (1, "p2a"), (1, "p2b"), (1, "p3"),
             (1, "p45"), (1, "p6"), (1, "p7")]

    for c, nm in order:
        sc_(c, nm)
        STAGE_FNS[nm](c)
    sc_(0, None)


_NC_CACHE = None


def build():
    global _NC_CACHE
    if _NC_CACHE is not None:
        return _NC_CACHE
    nc = bacc.Bacc(trn_type="TRN2", target_bir_lowering=False, debug=False,
                   enable_asserts=False)
    io = {}
    # all tensors are host-pre-arranged so every DMA is contiguous per
    # partition (8KB descriptors): W[.., p, ko*f] = W_orig[ko*128+p, f]
    shapes_bf = {
        "pk": [P, 11 * RPC],
        "W_eh": [2, P, KT * 512],
        "W_heads": [NH, 2, P, KT * 512],
        "W1": [2, P, KT * 512],
        "W2": [P, KT],
        "Wfa": [DEC_IN + 1, HID],
        "b_heads": [1, NH * HID],
    }
    shapes_f32 = {
        "bias_pp": [P, 3 * KT], "b2": [1],
    }
    for name, shp in shapes_bf.items():
        io[name] = nc.dram_tensor(name, shp, BF16, kind="ExternalInput").ap()
    for name, shp in shapes_f32.items():
        io[name] = nc.dram_tensor(name, shp, F32, kind="ExternalInput").ap()
    io["q"] = nc.dram_tensor("q", [RPC, 1], F32, kind="ExternalOutput").ap()

    from contextlib import ExitStack
    with tile.TileContext(nc) as tc, ExitStack() as ctx:
        _body(nc, tc, io, ctx)
    nc.compile()
    _NC_CACHE = nc
    return nc


def _ktile(w, nhalves):
    # [K, N] -> [nhalves, 128, KT*(N/nhalves)]: w_r[nh, p, ko*f] =
    # w[ko*128+p, nh*(N/nhalves)+f]
    K, N = w.shape
    nh = N // nhalves
    r = w.reshape(K // P, P, nhalves, nh).transpose(2, 1, 0, 3)
    return np.ascontiguousarray(r.reshape(nhalves, P, (K // P) * nh))


def _prep_inputs(inputs):
    import ml_dtypes
    bf16 = ml_dtypes.bfloat16
    arrs = {k: np.ascontiguousarray(np.asarray(v, dtype=np.float32))
            for k, v in inputs.items()}
    # host-side weight folding: dec_input feeds only decoder_H (no relu in
    # between), so W_fused = W_dec_in @ W_dh, b_fused = b_dec_in @ W_dh + b_dh
    wf = arrs["W_dec_in"] @ arrs["W_dh"]
    bfu = arrs["b_dec_in"] @ arrs["W_dh"] + arrs["b_dh"]
    wfa = np.ascontiguousarray(
        np.concatenate([wf, bfu[None, :]], axis=0)).astype(bf16)
    # s^T tiled as [128, 4*B]: sT[p, kt*B+r] = s[r, kt*128+p]
    sT = np.ascontiguousarray(
        arrs["s"].T.reshape(4, P, B).transpose(1, 0, 2)).astype(bf16)
    aT = np.ascontiguousarray(                # [129, B]: ones row appended
        np.concatenate([arrs["a"].T, np.ones((1, B), np.float32)],
                       axis=0)).astype(bf16)
    wcast = {
        "W_eh": _ktile(arrs["W_eh"], 2).astype(bf16),
        "W_heads": np.ascontiguousarray(np.stack(
            [_ktile(arrs["W_heads"][h], 2) for h in range(NH)])).astype(bf16),
        "W1": _ktile(arrs["W1"], 2).astype(bf16),
        "W2": np.ascontiguousarray(
            arrs["W2"].reshape(KT, P).T).astype(bf16),
        "Wfa": wfa,
    }
    wenc_t = _ktile(arrs["W_enc_in"][0:512], 1)[0].astype(bf16)  # [P, 4096]
    wencr = arrs["W_enc_in"][512:544].astype(bf16)               # [32, 1024]
    bcast = {
        "bias_pp": np.ascontiguousarray(np.concatenate(
            [arrs[k].reshape(KT, P).T for k in ("b_enc_in", "b_eh", "b1")],
            axis=1)),
        "b_heads": np.ascontiguousarray(
            arrs["b_heads"].reshape(1, NH * HID)).astype(bf16),
        "b2": arrs["b2"],
    }
    in_maps = []
    for c in range(NCORES):
        m = dict(bcast)
        m.update(wcast)
        # packed startup tensor: [wenc | sT_c0 | wencr | aown_c0 | aoth_c0 |
        # sT_c1 | aown_c1 | aoth_c1]
        pkm = np.zeros((P, 11264), bf16)
        pkm[:, 0:4096] = wenc_t
        pkm[0:ENC_REM, 6144:7168] = wencr
        for ch in range(2):
            r0 = c * RPC + ch * CH
            o = 4096 if ch == 0 else 8192
            oa = 7168 if ch == 0 else 10240
            pkm[:, o:o + 2048] = sT[:, :, r0:r0 + CH].reshape(P, 4 * CH)
            aTc = aT[:, r0:r0 + CH]
            pkm[0:ACTD, oa:oa + CH] = aTc[0:ACTD]
            pkm[0:DEC_IN + 1, oa + CH:oa + 2 * CH] = aTc[ACTD:P + 1]
        m["pk"] = pkm
        in_maps.append(m)
    return in_maps


def run(inputs, trace=False):
    from concourse.bass_utils import run_bass_kernel_spmd
    nc = build()
    in_maps = _prep_inputs(inputs)
    res = run_bass_kernel_spmd(nc, in_maps, core_ids=list(range(NCORES)),
                               trace=trace)
    q = np.concatenate([r["q"] for r in res.results], axis=0)
    return np.ascontiguousarray(q.astype(np.float32)), res


def kernel(**inputs) -> np.ndarray:
    q, _ = run(inputs, trace=False)
    return q


# revision 62
# speedup vs baseline: 1.0642x; 1.0230x over previous
"""Trainium2 Bass kernel for nn_ATT_critic (attention critic network).

Strategy: data-parallel over batch across 8 NeuronCores (1024 rows/core),
2 chunks of 512 rows per core; all big GEMMs on the PE in bf16 (PSUM
accumulation in fp32).

Key design points:
  - host-side weight folding: W_fused = W_dec_in @ W_dh (+ fused bias) is a
    weight-only precompute, done once on the host and shipped augmented with
    its bias as a [97, HID] tensor (ones-row trick).
  - host-side transposes + bf16 cast: s^T and a^T are passed per-core in
    bf16 so the kernel DMAs activations directly in [feature, row] layout;
    all weights are pre-cast to bf16 (the PE's fp32r mode rounds to
    bf16-level precision anyway, so this costs ~nothing numerically and
    halves all weight DMA traffic, which was the p3 bottleneck).
  - bias via PSUM prefill: the heads-layer biases are partition-replicated
    once (GpSimd partition_broadcast) and copied into PSUM before each
    accumulation group (alternating DVE/ACT), replacing 128 K=1 ones-row
    bias matmuls (322 ns each on the PE).
  - p2b bias via K-augmentation (97-row stationary with a host-side ones
    row in aT).
  - emission order overlaps chunk 1's input DMA + enc GEMM with chunk 0's
    softmax/context tail; softmax normalization is interleaved with the
    context transposes per row-tile; the weight pool is quad-buffered so
    upcoming layers' W tiles prefetch during the heads GEMM.
"""

import numpy as np

import concourse.bass as bass
import concourse.tile as tile
from concourse import mybir
from concourse import bacc
from concourse.masks import make_identity

P = 128
B = 8192
NCORES = 8
RPC = B // NCORES        # rows per core
CH = 512                 # rows per chunk
NCHUNK = RPC // CH
MT = CH // P             # row tiles per chunk
HID = 1024
KT = HID // P            # k tiles over hidden dim
NH = 8                   # heads
ACTD = 32
DEC_IN = 96
ENC_REM = 32             # 544 - 512

F32 = mybir.dt.float32
BF16 = mybir.dt.bfloat16
AF = mybir.ActivationFunctionType
ALU = mybir.AluOpType
AX = mybir.AxisListType

F32_WEIGHTS = ["b_enc_in", "b_eh", "b_heads", "b1", "b2"]
BF16_WEIGHTS = ["W_enc_in", "W_eh", "W_heads", "W1", "W2"]


def _body(nc, tc, io, ctx):
    q_ap = io["q"]

    const = ctx.enter_context(tc.tile_pool(name="const", bufs=1))
    acts = ctx.enter_context(tc.tile_pool(name="acts", bufs=1))
    wp = ctx.enter_context(tc.tile_pool(name="wp", bufs=5))
    ps = ctx.enter_context(tc.tile_pool(name="ps", bufs=1, space="PSUM"))

    def wtile(shape, name):
        return wp.tile(shape, BF16, tag="w", bufs=5, name=name)

    def t8tile(shape, name, dtype=BF16):
        return acts.tile(shape, dtype, tag="t8", bufs=3, name=name)

    def junk(shape, dtype, name):
        return acts.tile(shape, dtype, tag="junk", bufs=2, name=name)

    def psmm(name, shape=None):
        return ps.tile(shape or [P, 512], F32, tag="mm", bufs=4, name=name)

    def pstr(name, dtype=F32):
        return ps.tile([P, 512], dtype, tag="tr", bufs=2, name=name)

    def psq(name):
        return ps.tile([1, 512], F32, tag="q", bufs=2, name=name)

    # ---------------- constants / one-time init ----------------
    identity_bf = const.tile([P, P], BF16, name="identity_bf")
    make_identity(nc, identity_bf)

    # [b_enc | b_eh | b1] merged into one DMA (fewer descriptors: descriptor
    # generation throughput is the startup bottleneck)
    bias_pp = const.tile([P, 3 * KT], F32, name="bias_pp")
    nc.sync.dma_start(bias_pp, io["bias_pp"])
    b_enc_pp = bias_pp[:, 0:KT]
    b_eh_pp = bias_pp[:, KT:2 * KT]
    b1_pp = bias_pp[:, 2 * KT:3 * KT]
    b2sb = const.tile([1, 1], F32, name="b2sb")
    nc.sync.dma_start(b2sb, io["b2"][None, :])

    b_full = const.tile([P, NH, HID], BF16, name="b_full")

    def binit():
        # head biases (host-cast bf16, riding in the packed tensor on
        # partition 0) replicated across all partitions via GpSimd
        # partition_broadcast
        for h in range(NH):
            nc.gpsimd.partition_broadcast(
                b_full[:, h, :], pk[0:1, 12296 + h * HID:12296 + (h + 1) * HID])

    # ---------------- per-chunk stages ----------------
    _scope_stack = [None]

    def sc_(c, nm):
        prev = _scope_stack.pop()
        if prev is not None:
            nc.leave_named_scope(prev[0], prev[1], False)
        if nm is not None:
            full = f"c{c}_{nm}"
            sid, _ = nc.enter_named_scope(full, False)
            _scope_stack.append((full, sid))
        else:
            _scope_stack.append(None)

    S = [dict() for _ in range(NCHUNK)]

    # all startup-critical tensors (W_enc tiled + remainder, s^T tiled, a^T
    # with its ones row) ride in host-packed DMAs: descriptor generation
    # throughput, not bandwidth, limits the startup, so 128 fat descriptors
    # beat ~450 small ones. Two pieces: chunk 0's (+ weights) is critical,
    # chunk 1's arrives while chunk 0 computes.
    PKW = 20488
    PK_CRIT = 8192
    pk = const.tile([P, PKW], BF16, name="pk")
    nc.sync.dma_start(pk[:, 0:PK_CRIT], io["pk"][:, 0:PK_CRIT])
    nc.sync.dma_start(pk[:, PK_CRIT:PKW], io["pk"][:, PK_CRIT:PKW])
    full = dict(
        wenc=pk[:, 0:4096].rearrange("p (ko f) -> p ko f", ko=4),
        wencr=pk[0:ENC_REM, 6144:7168],
    )
    W2sb = pk[:, 11264:11272]
    wfa_sb = pk[0:DEC_IN + 1, 11272:12296]
    _pkc = [dict(sT=pk[:, 4096:6144].rearrange("p (kt r) -> p kt r", kt=4),
                 aownT=pk[0:ACTD, 7168:7680],
                 aothT=pk[0:DEC_IN + 1, 7680:8192]),
            dict(sT=pk[:, 8192:10240].rearrange("p (kt r) -> p kt r", kt=4),
                 aownT=pk[0:ACTD, 10240:10752],
                 aothT=pk[0:DEC_IN + 1, 10752:11264])]

    def p1(c):
        S[c].update(_pkc[c])

    def p1b(c):
        st = S[c]
        wenc = full["wenc"]
        enc_inT = t8tile([P, KT, CH], "enc_inT")
        for m in range(KT):
            pm = psmm("pm_enc")
            for kt in range(4):
                nc.tensor.matmul(pm, lhsT=wenc[:, kt, m * P:(m + 1) * P],
                                 rhs=st["sT"][:, kt, :], start=(kt == 0),
                                 stop=False)
            nc.tensor.matmul(pm, lhsT=full["wencr"][:, m * P:(m + 1) * P],
                             rhs=st["aownT"], start=False, stop=True)
            nc.scalar.activation(enc_inT[:, m, :], pm, AF.Identity,
                                 bias=b_enc_pp[:, m:m + 1])
        st["enc_inT"] = enc_inT

    def p2a(c):
        st = S[c]
        encHT = t8tile([P, KT, CH], "encHT")
        for mh in range(2):
            weh = wtile([P, KT, 512], "weh")
            nc.sync.dma_start(
                weh, io["W_eh"][mh].rearrange("p (ko f) -> p ko f", ko=KT))
            for mi in range(4):
                m = mh * 4 + mi
                pm = psmm("pm_eh")
                for kt in range(KT):
                    nc.tensor.matmul(pm, lhsT=weh[:, kt, mi * P:(mi + 1) * P],
                                     rhs=st["enc_inT"][:, kt, :],
                                     start=(kt == 0), stop=(kt == KT - 1))
                nc.scalar.activation(encHT[:, m, :], pm, AF.Relu,
                                     bias=b_eh_pp[:, m:m + 1])
        st["encHT"] = encHT

    def p2b(c):
        # DH = relu([a_others, 1] @ [W_fused; b_fused]) : K=97, no bias matmul
        st = S[c]
        DH = acts.tile([P, MT, HID], BF16, tag="dh", bufs=1, name="DH")
        for mt in range(MT):
            for n in range(2):
                pm = psmm("pm_dh")
                nc.tensor.matmul(pm,
                                 lhsT=st["aothT"][:, mt * P:(mt + 1) * P],
                                 rhs=wfa_sb[:, n * 512:(n + 1) * 512],
                                 start=True, stop=True)
                nc.scalar.activation(DH[:, mt, n * 512:(n + 1) * 512], pm,
                                     AF.Relu)
        st["DH"] = DH

    def p3(c):
        st = S[c]
        EH = acts.tile([P, MT, NH, HID], BF16, tag="eh", bufs=1, name="EH")
        scores = acts.tile([P, MT, NH], F32, tag="scores", bufs=2, name="scores")

        def emit_group(h, n, mt, whn):
            pm = psmm("pm_hd")
            # bias prefill from the replicated tile (alternating DVE/ACT,
            # off the PE critical path); matmuls accumulate on top.
            # GpSimd cannot write PSUM on TRN2.
            if mt == 0:
                nc.vector.tensor_copy(pm, b_full[:, h, n * 512:(n + 1) * 512])
            else:
                nc.scalar.activation(pm, b_full[:, h, n * 512:(n + 1) * 512],
                                     AF.Copy)
            for kt in range(KT):
                nc.tensor.matmul(
                    pm, lhsT=st["encHT"][:, kt, mt * P:(mt + 1) * P],
                    rhs=whn[:, kt, :], start=False,
                    stop=(kt == KT - 1), skip_group_check=True)
            nc.scalar.activation(EH[:, mt, h, n * 512:(n + 1) * 512],
                                 pm, AF.Relu)

        for h in range(NH):
            whns = []
            for n in range(2):
                whn = wtile([P, KT, 512], f"whn{h}_{n}")
                nc.sync.dma_start(
                    whn, io["W_heads"][h, n].rearrange("p (ko f) -> p ko f",
                                                       ko=KT))
                whns.append(whn)
                if h < NH - 1:
                    for mt in range(MT):
                        emit_group(h, n, mt, whn)
            def ctx_accum(hp, mt):
                # head hp's scores are complete: fold its (unnormalized)
                # softmax contribution into the context, spread evenly so
                # the DVE queue never floods and delays the PSUM bias
                # prefills. exp without max-subtraction is safe: scores ~< 12.
                nc.scalar.activation(exps[:, mt, hp:hp + 1],
                                     scores[:, mt, hp:hp + 1], AF.Exp)
                if hp == 0:
                    nc.vector.tensor_scalar_mul(ctx_t[:, mt, :],
                                                EH[:, mt, 0, :],
                                                exps[:, mt, 0:1])
                else:
                    nc.vector.scalar_tensor_tensor(
                        out=ctx_t[:, mt, :], in0=EH[:, mt, hp, :],
                        scalar=exps[:, mt, hp:hp + 1],
                        in1=ctx_t[:, mt, :], op0=ALU.mult, op1=ALU.add)

            if h == 1:
                exps = acts.tile([P, MT, NH], F32, tag="attn", bufs=2,
                                 name="exps")
                ctx_t = t8tile([P, MT, HID], "ctx_t")
                st["stats"] = acts.tile([P, MT, 2], F32, tag="stats",
                                        bufs=2, name="stats")
            if 1 <= h < NH - 1:
                for mt in range(MT):
                    ctx_accum(h - 1, mt)
            for mt in range(MT):
                if h == NH - 1:
                    # previous head's context fold-in rides per row-tile here
                    # so the DVE burst never queues ahead of this head's
                    # PSUM prefills
                    ctx_accum(h - 1, mt)
                    # last head: both n-halves of this row-tile back-to-back,
                    # then its scores + softmax tail immediately — the tail
                    # for row-tile mt overlaps row-tile mt+1's matmuls, so
                    # almost nothing drains after the PE's last group.
                    emit_group(h, 0, mt, whns[0])
                    emit_group(h, 1, mt, whns[1])
                # scores[:, mt, h] = rowsum(EH_h * DH): single fused DVE op
                # (multiply with free-dim accumulation side-output)
                jt = junk([P, HID], BF16, "jsc")
                nc.vector.scalar_tensor_tensor(
                    out=jt[:, :], in0=EH[:, mt, h, :], scalar=1.0,
                    in1=st["DH"][:, mt, :], op0=ALU.bypass, op1=ALU.mult,
                    accum_out=scores[:, mt, h:h + 1])
                if h == NH - 1:
                    stats = st["stats"]
                    nc.scalar.activation(exps[:, mt, h:h + 1],
                                         scores[:, mt, h:h + 1], AF.Exp)
                    nc.vector.scalar_tensor_tensor(
                        out=ctx_t[:, mt, :], in0=EH[:, mt, h, :],
                        scalar=exps[:, mt, h:h + 1],
                        in1=ctx_t[:, mt, :], op0=ALU.mult, op1=ALU.add)
                    sumexp = stats[:, mt, 0:1]
                    rsum = stats[:, mt, 1:2]
                    nc.vector.tensor_reduce(sumexp, exps[:, mt, :],
                                            axis=AX.X, op=ALU.add)
                    nc.vector.reciprocal(rsum, sumexp)
                    nc.vector.tensor_scalar_mul(ctx_t[:, mt, :],
                                                ctx_t[:, mt, :], rsum)
        st.update(EH=EH, scores=scores, exps=exps, ctx_t=ctx_t)

    def p45(c):
        # context transpose to [feature, row] for fc1; the softmax 1/sumexp
        # normalization rides along for free as a diag matrix in place of
        # the transpose identity
        st = S[c]
        ctx_t = st["ctx_t"]
        ctxT = t8tile([P, KT, CH], "ctxT")
        for mt in range(MT):
            for g in range(2):
                trp = pstr("trc", dtype=BF16)
                for ft in range(4):
                    nc.tensor.transpose(
                        trp[:, ft * P:(ft + 1) * P],
                        ctx_t[:, mt, (g * 4 + ft) * P:(g * 4 + ft + 1) * P],
                        identity_bf)
                nc.vector.tensor_copy(
                    ctxT[:, g * 4:(g + 1) * 4, mt * P:(mt + 1) * P],
                    trp.rearrange("p (ft x) -> p ft x", ft=4))
        st["ctxT"] = ctxT

    def p6(c):
        st = S[c]
        x1T = t8tile([P, KT, CH], "x1T")
        pq = psq("pq")
        for mh in range(2):
            w1 = wtile([P, KT, 512], "w1t")
            nc.sync.dma_start(
                w1, io["W1"][mh].rearrange("p (ko f) -> p ko f", ko=KT))
            for mi in range(4):
                m = mh * 4 + mi
                pm = psmm("pm_fc1")
                for kt in range(KT):
                    nc.tensor.matmul(pm, lhsT=w1[:, kt, mi * P:(mi + 1) * P],
                                     rhs=st["ctxT"][:, kt, :],
                                     start=(kt == 0), stop=(kt == KT - 1))
                nc.scalar.activation(x1T[:, m, :], pm, AF.Relu,
                                     bias=b1_pp[:, m:m + 1])
            # fc2 rides behind each half's evictions so the final output
            # eviction starts right after the last fc1 matmul
            for kt in range(mh * 4, mh * 4 + 4):
                nc.tensor.matmul(pq, lhsT=W2sb[:, kt:kt + 1],
                                 rhs=x1T[:, kt, :],
                                 start=(kt == 0), stop=(kt == KT - 1))
        st["pq"] = pq

    def p7(c):
        st = S[c]
        r0 = c * CH
        q_rowT = acts.tile([1, CH], F32, tag="q_rowT", bufs=1, name="q_rowT")
        nc.scalar.activation(q_rowT[0:1, :], st["pq"][0:1, :], AF.Identity,
                             bias=b2sb[0:1, 0:1])
        nc.sync.dma_start(q_ap[r0:r0 + CH, 0][None, :], q_rowT[0:1, :])

    STAGE_FNS = {"binit": lambda c: binit(), "p1": p1, "p1b": p1b,
                 "p2a": p2a, "p2b": p2b, "p3": p3, "p45": p45, "p6": p6,
                 "p7": p7}

    # Emission order: chunk 1's enc GEMMs are emitted inside chunk 0's
    # softmax tail so the PE stays busy while the DVE finishes the last
    # head / normalization.
    order = [(0, "binit"), (0, "p1"), (1, "p1"),
             (0, "p1b"), (0, "p2a"), (0, "p2b"), (0, "p3"),
             (1, "p1b"),
             (0, "p45"), (0, "p6"), (0, "p7"),
             (1, "p2a"), (1, "p2b"), (1, "p3"),
             (1, "p45"), (1, "p6"), (1, "p7")]

    for c, nm in order:
        sc_(c, nm)
        STAGE_FNS[nm](c)
    sc_(0, None)


_NC_CACHE = None


def build():
    global _NC_CACHE
    if _NC_CACHE is not None:
        return _NC_CACHE
    nc = bacc.Bacc(trn_type="TRN2", target_bir_lowering=False, debug=False,
                   enable_asserts=False)
    io = {}
    # all tensors are host-pre-arranged so every DMA is contiguous per
    # partition (8KB descriptors): W[.., p, ko*f] = W_orig[ko*128+p, f]
    shapes_bf = {
        "pk": [P, 20488],
        "W_eh": [2, P, KT * 512],
        "W_heads": [NH, 2, P, KT * 512],
        "W1": [2, P, KT * 512],
    }
    shapes_f32 = {
        "bias_pp": [P, 3 * KT], "b2": [1],
    }
    for name, shp in shapes_bf.items():
        io[name] = nc.dram_tensor(name, shp, BF16, kind="ExternalInput").ap()
    for name, shp in shapes_f32.items():
        io[name] = nc.dram_tensor(name, shp, F32, kind="ExternalInput").ap()
    io["q"] = nc.dram_tensor("q", [RPC, 1], F32, kind="ExternalOutput").ap()

    from contextlib import ExitStack
    with tile.TileContext(nc) as tc, ExitStack() as ctx:
        _body(nc, tc, io, ctx)
    nc.compile()
    _NC_CACHE = nc
    return nc


def _ktile(w, nhalves):
    # [K, N] -> [nhalves, 128, KT*(N/nhalves)]: w_r[nh, p, ko*f] =
    # w[ko*128+p, nh*(N/nhalves)+f]
    K, N = w.shape
    nh = N // nhalves
    r = w.reshape(K // P, P, nhalves, nh).transpose(2, 1, 0, 3)
    return np.ascontiguousarray(r.reshape(nhalves, P, (K // P) * nh))


def _prep_inputs(inputs):
    import ml_dtypes
    bf16 = ml_dtypes.bfloat16
    arrs = {k: np.ascontiguousarray(np.asarray(v, dtype=np.float32))
            for k, v in inputs.items()}
    # host-side weight folding: dec_input feeds only decoder_H (no relu in
    # between), so W_fused = W_dec_in @ W_dh, b_fused = b_dec_in @ W_dh + b_dh
    wf = arrs["W_dec_in"] @ arrs["W_dh"]
    bfu = arrs["b_dec_in"] @ arrs["W_dh"] + arrs["b_dh"]
    wfa = np.ascontiguousarray(
        np.concatenate([wf, bfu[None, :]], axis=0)).astype(bf16)
    # s^T tiled as [128, 4*B]: sT[p, kt*B+r] = s[r, kt*128+p]
    sT = np.ascontiguousarray(
        arrs["s"].T.reshape(4, P, B).transpose(1, 0, 2)).astype(bf16)
    aT = np.ascontiguousarray(                # [129, B]: ones row appended
        np.concatenate([arrs["a"].T, np.ones((1, B), np.float32)],
                       axis=0)).astype(bf16)
    wcast = {
        "W_eh": _ktile(arrs["W_eh"], 2).astype(bf16),
        "W_heads": np.ascontiguousarray(np.stack(
            [_ktile(arrs["W_heads"][h], 2) for h in range(NH)])).astype(bf16),
        "W1": _ktile(arrs["W1"], 2).astype(bf16),
    }
    wenc_t = _ktile(arrs["W_enc_in"][0:512], 1)[0].astype(bf16)  # [P, 4096]
    wencr = arrs["W_enc_in"][512:544].astype(bf16)               # [32, 1024]
    bcast = {
        "bias_pp": np.ascontiguousarray(np.concatenate(
            [arrs[k].reshape(KT, P).T for k in ("b_enc_in", "b_eh", "b1")],
            axis=1)),
        "b2": arrs["b2"],
    }
    in_maps = []
    for c in range(NCORES):
        m = dict(bcast)
        m.update(wcast)
        # packed startup tensor: [wenc | sT_c0 | wencr | aown_c0 | aoth_c0 |
        # sT_c1 | aown_c1 | aoth_c1 | W2 | Wfa | b_heads]
        pkm = np.zeros((P, 20488), bf16)
        pkm[:, 0:4096] = wenc_t
        pkm[0:ENC_REM, 6144:7168] = wencr
        for ch in range(2):
            r0 = c * RPC + ch * CH
            o = 4096 if ch == 0 else 8192
            oa = 7168 if ch == 0 else 10240
            pkm[:, o:o + 2048] = sT[:, :, r0:r0 + CH].reshape(P, 4 * CH)
            aTc = aT[:, r0:r0 + CH]
            pkm[0:ACTD, oa:oa + CH] = aTc[0:ACTD]
            pkm[0:DEC_IN + 1, oa + CH:oa + 2 * CH] = aTc[ACTD:P + 1]
        pkm[:, 11264:11272] = arrs["W2"].reshape(KT, P).T.astype(bf16)
        pkm[0:DEC_IN + 1, 11272:12296] = wfa
        pkm[0:1, 12296:20488] = arrs["b_heads"].reshape(1, -1).astype(bf16)
        m["pk"] = pkm
        in_maps.append(m)
    return in_maps


def run(inputs, trace=False):
    from concourse.bass_utils import run_bass_kernel_spmd
    nc = build()
    in_maps = _prep_inputs(inputs)
    res = run_bass_kernel_spmd(nc, in_maps, core_ids=list(range(NCORES)),
                               trace=trace)
    q = np.concatenate([r["q"] for r in res.results], axis=0)
    return np.ascontiguousarray(q.astype(np.float32)), res


def kernel(**inputs) -> np.ndarray:
    q, _ = run(inputs, trace=False)
    return q


# revision 64
# speedup vs baseline: 1.0708x; 1.0063x over previous
"""Trainium2 Bass kernel for nn_ATT_critic (attention critic network).

Strategy: data-parallel over batch across 8 NeuronCores (1024 rows/core),
2 chunks of 512 rows per core; all big GEMMs on the PE in bf16 (PSUM
accumulation in fp32).

Key design points:
  - host-side weight folding: W_fused = W_dec_in @ W_dh (+ fused bias) is a
    weight-only precompute, done once on the host and shipped augmented with
    its bias as a [97, HID] tensor (ones-row trick).
  - host-side transposes + bf16 cast: s^T and a^T are passed per-core in
    bf16 so the kernel DMAs activations directly in [feature, row] layout;
    all weights are pre-cast to bf16 (the PE's fp32r mode rounds to
    bf16-level precision anyway, so this costs ~nothing numerically and
    halves all weight DMA traffic, which was the heads-GEMM bottleneck).
  - all weights land host-pre-tiled so every DMA is contiguous per
    partition (8KB descriptors); the startup-critical set (W_enc, s^T/a^T
    chunk 0) plus the small constants ride in one packed tensor, because
    descriptor-generation throughput, not bandwidth, limits the startup.
  - bias via PSUM prefill: the heads-layer biases are partition-replicated
    once (GpSimd partition_broadcast) and copied into PSUM before each
    accumulation group (alternating DVE/ACT), replacing 128 K=1 ones-row
    bias matmuls (322 ns each on the PE).
  - p2b bias via K-augmentation (97-row stationary with a host-side ones
    row in aT).
  - scores come from a single fused DVE op per (row-tile, head) (multiply
    with free-dim accumulate side-output); each head's softmax contribution
    folds into the context right after its scores complete, so the DVE
    queue never floods; the last head interleaves its two N-halves per
    row-tile so the softmax tail drains under the remaining matmuls.
  - emission order overlaps chunk 1's enc GEMM with chunk 0's softmax
    tail; the weight pool is 5-deep so upcoming layers' W tiles prefetch
    during the heads GEMM; fc2 accumulates behind fc1's evictions.
"""

import numpy as np

import concourse.bass as bass
import concourse.tile as tile
from concourse import mybir
from concourse import bacc
from concourse.masks import make_identity

P = 128
B = 8192
NCORES = 8
RPC = B // NCORES        # rows per core
CH = 512                 # rows per chunk
NCHUNK = RPC // CH
MT = CH // P             # row tiles per chunk
HID = 1024
KT = HID // P            # k tiles over hidden dim
NH = 8                   # heads
ACTD = 32
DEC_IN = 96
ENC_REM = 32             # 544 - 512

F32 = mybir.dt.float32
BF16 = mybir.dt.bfloat16
AF = mybir.ActivationFunctionType
ALU = mybir.AluOpType
AX = mybir.AxisListType

def _body(nc, tc, io, ctx):
    q_ap = io["q"]

    const = ctx.enter_context(tc.tile_pool(name="const", bufs=1))
    acts = ctx.enter_context(tc.tile_pool(name="acts", bufs=1))
    wp = ctx.enter_context(tc.tile_pool(name="wp", bufs=5))
    ps = ctx.enter_context(tc.tile_pool(name="ps", bufs=1, space="PSUM"))

    def wtile(shape, name):
        return wp.tile(shape, BF16, tag="w", bufs=5, name=name)

    def t8tile(shape, name, dtype=BF16):
        return acts.tile(shape, dtype, tag="t8", bufs=3, name=name)

    def junk(shape, dtype, name):
        return acts.tile(shape, dtype, tag="junk", bufs=2, name=name)

    def psmm(name, shape=None):
        return ps.tile(shape or [P, 512], F32, tag="mm", bufs=4, name=name)

    def pstr(name, dtype=F32):
        return ps.tile([P, 512], dtype, tag="tr", bufs=2, name=name)

    def psq(name):
        return ps.tile([1, 512], F32, tag="q", bufs=2, name=name)

    # ---------------- constants / one-time init ----------------
    identity_bf = const.tile([P, P], BF16, name="identity_bf")
    make_identity(nc, identity_bf)

    # [b_enc | b_eh | b1] merged into one DMA (fewer descriptors: descriptor
    # generation throughput is the startup bottleneck)
    bias_pp = const.tile([P, 3 * KT], F32, name="bias_pp")
    nc.sync.dma_start(bias_pp, io["bias_pp"])
    b_enc_pp = bias_pp[:, 0:KT]
    b_eh_pp = bias_pp[:, KT:2 * KT]
    b1_pp = bias_pp[:, 2 * KT:3 * KT]
    b2sb = const.tile([1, 1], F32, name="b2sb")
    nc.sync.dma_start(b2sb, io["b2"][None, :])

    b_full = const.tile([P, NH, HID], BF16, name="b_full")

    def binit():
        # head biases (host-cast bf16, riding in the packed tensor on
        # partition 0) replicated across all partitions via GpSimd
        # partition_broadcast
        for h in range(NH):
            nc.gpsimd.partition_broadcast(
                b_full[:, h, :], pk[0:1, 12296 + h * HID:12296 + (h + 1) * HID])

    # ---------------- per-chunk stages ----------------
    _scope_stack = [None]

    def sc_(c, nm):
        prev = _scope_stack.pop()
        if prev is not None:
            nc.leave_named_scope(prev[0], prev[1], False)
        if nm is not None:
            full = f"c{c}_{nm}"
            sid, _ = nc.enter_named_scope(full, False)
            _scope_stack.append((full, sid))
        else:
            _scope_stack.append(None)

    S = [dict() for _ in range(NCHUNK)]

    # all startup-critical tensors (W_enc tiled + remainder, s^T tiled, a^T
    # with its ones row) ride in host-packed DMAs: descriptor generation
    # throughput, not bandwidth, limits the startup, so 128 fat descriptors
    # beat ~450 small ones. Two pieces: chunk 0's (+ weights) is critical,
    # chunk 1's arrives while chunk 0 computes.
    PKW = 20488
    PK_CRIT = 8192
    pk = const.tile([P, PKW], BF16, name="pk")
    nc.sync.dma_start(pk[:, 0:PK_CRIT], io["pk"][:, 0:PK_CRIT])
    nc.sync.dma_start(pk[:, PK_CRIT:PKW], io["pk"][:, PK_CRIT:PKW])
    full = dict(
        wenc=pk[:, 0:4096].rearrange("p (ko f) -> p ko f", ko=4),
        wencr=pk[0:ENC_REM, 6144:7168],
    )
    W2sb = pk[:, 11264:11272]
    wfa_sb = pk[0:DEC_IN + 1, 11272:12296]
    _pkc = [dict(sT=pk[:, 4096:6144].rearrange("p (kt r) -> p kt r", kt=4),
                 aownT=pk[0:ACTD, 7168:7680],
                 aothT=pk[0:DEC_IN + 1, 7680:8192]),
            dict(sT=pk[:, 8192:10240].rearrange("p (kt r) -> p kt r", kt=4),
                 aownT=pk[0:ACTD, 10240:10752],
                 aothT=pk[0:DEC_IN + 1, 10752:11264])]

    def p1(c):
        S[c].update(_pkc[c])

    def p1b(c):
        st = S[c]
        wenc = full["wenc"]
        enc_inT = t8tile([P, KT, CH], "enc_inT")
        for m in range(KT):
            pm = psmm("pm_enc")
            for kt in range(4):
                nc.tensor.matmul(pm, lhsT=wenc[:, kt, m * P:(m + 1) * P],
                                 rhs=st["sT"][:, kt, :], start=(kt == 0),
                                 stop=False)
            nc.tensor.matmul(pm, lhsT=full["wencr"][:, m * P:(m + 1) * P],
                             rhs=st["aownT"], start=False, stop=True)
            nc.scalar.activation(enc_inT[:, m, :], pm, AF.Identity,
                                 bias=b_enc_pp[:, m:m + 1])
        st["enc_inT"] = enc_inT

    def p2a(c):
        st = S[c]
        encHT = t8tile([P, KT, CH], "encHT")
        for mh in range(2):
            weh = wtile([P, KT, 512], "weh")
            nc.sync.dma_start(
                weh, io["W_eh"][mh].rearrange("p (ko f) -> p ko f", ko=KT))
            for mi in range(4):
                m = mh * 4 + mi
                pm = psmm("pm_eh")
                for kt in range(KT):
                    nc.tensor.matmul(pm, lhsT=weh[:, kt, mi * P:(mi + 1) * P],
                                     rhs=st["enc_inT"][:, kt, :],
                                     start=(kt == 0), stop=(kt == KT - 1))
                nc.scalar.activation(encHT[:, m, :], pm, AF.Relu,
                                     bias=b_eh_pp[:, m:m + 1])
        st["encHT"] = encHT

    def p2b(c):
        # DH = relu([a_others, 1] @ [W_fused; b_fused]) : K=97, no bias matmul
        st = S[c]
        DH = acts.tile([P, MT, HID], BF16, tag="dh", bufs=1, name="DH")
        for mt in range(MT):
            for n in range(2):
                pm = psmm("pm_dh")
                nc.tensor.matmul(pm,
                                 lhsT=st["aothT"][:, mt * P:(mt + 1) * P],
                                 rhs=wfa_sb[:, n * 512:(n + 1) * 512],
                                 start=True, stop=True)
                nc.scalar.activation(DH[:, mt, n * 512:(n + 1) * 512], pm,
                                     AF.Relu)
        st["DH"] = DH

    def p3(c):
        st = S[c]
        EH = acts.tile([P, MT, NH, HID], BF16, tag="eh", bufs=1, name="EH")
        scores = acts.tile([P, MT, NH], F32, tag="scores", bufs=2, name="scores")

        def emit_group(h, n, mt, whn):
            pm = psmm("pm_hd")
            # bias prefill from the replicated tile (alternating DVE/ACT,
            # off the PE critical path); matmuls accumulate on top.
            # GpSimd cannot write PSUM on TRN2.
            if mt == 0:
                nc.vector.tensor_copy(pm, b_full[:, h, n * 512:(n + 1) * 512])
            else:
                nc.scalar.activation(pm, b_full[:, h, n * 512:(n + 1) * 512],
                                     AF.Copy)
            for kt in range(KT):
                nc.tensor.matmul(
                    pm, lhsT=st["encHT"][:, kt, mt * P:(mt + 1) * P],
                    rhs=whn[:, kt, :], start=False,
                    stop=(kt == KT - 1), skip_group_check=True)
            nc.scalar.activation(EH[:, mt, h, n * 512:(n + 1) * 512],
                                 pm, AF.Relu)

        for h in range(NH):
            whns = []
            for n in range(2):
                whn = wtile([P, KT, 512], f"whn{h}_{n}")
                nc.sync.dma_start(
                    whn, io["W_heads"][h, n].rearrange("p (ko f) -> p ko f",
                                                       ko=KT))
                whns.append(whn)
                if h < NH - 1:
                    for mt in range(MT):
                        emit_group(h, n, mt, whn)
            def ctx_accum(hp, mt):
                # head hp's scores are complete: fold its (unnormalized)
                # softmax contribution into the context, spread evenly so
                # the DVE queue never floods and delays the PSUM bias
                # prefills. exp without max-subtraction is safe: scores ~< 12.
                nc.scalar.activation(exps[:, mt, hp:hp + 1],
                                     scores[:, mt, hp:hp + 1], AF.Exp)
                if hp == 0:
                    nc.vector.tensor_scalar_mul(ctx_t[:, mt, :],
                                                EH[:, mt, 0, :],
                                                exps[:, mt, 0:1])
                else:
                    nc.vector.scalar_tensor_tensor(
                        out=ctx_t[:, mt, :], in0=EH[:, mt, hp, :],
                        scalar=exps[:, mt, hp:hp + 1],
                        in1=ctx_t[:, mt, :], op0=ALU.mult, op1=ALU.add)

            if h == 1:
                exps = acts.tile([P, MT, NH], F32, tag="attn", bufs=2,
                                 name="exps")
                ctx_t = t8tile([P, MT, HID], "ctx_t")
                st["stats"] = acts.tile([P, MT, 2], F32, tag="stats",
                                        bufs=2, name="stats")
            if 1 <= h < NH - 1:
                for mt in range(MT):
                    ctx_accum(h - 1, mt)
            for mt in range(MT):
                if h == NH - 1:
                    # previous head's context fold-in rides per row-tile here
                    # so the DVE burst never queues ahead of this head's
                    # PSUM prefills
                    ctx_accum(h - 1, mt)
                    # last head: both n-halves of this row-tile back-to-back,
                    # then its scores + softmax tail immediately — the tail
                    # for row-tile mt overlaps row-tile mt+1's matmuls, so
                    # almost nothing drains after the PE's last group.
                    emit_group(h, 0, mt, whns[0])
                    emit_group(h, 1, mt, whns[1])
                # scores[:, mt, h] = rowsum(EH_h * DH): single fused DVE op
                # (multiply with free-dim accumulation side-output)
                jt = junk([P, HID], BF16, "jsc")
                nc.vector.scalar_tensor_tensor(
                    out=jt[:, :], in0=EH[:, mt, h, :], scalar=1.0,
                    in1=st["DH"][:, mt, :], op0=ALU.bypass, op1=ALU.mult,
                    accum_out=scores[:, mt, h:h + 1])
                if h == NH - 1:
                    stats = st["stats"]
                    nc.scalar.activation(exps[:, mt, h:h + 1],
                                         scores[:, mt, h:h + 1], AF.Exp)
                    nc.vector.scalar_tensor_tensor(
                        out=ctx_t[:, mt, :], in0=EH[:, mt, h, :],
                        scalar=exps[:, mt, h:h + 1],
                        in1=ctx_t[:, mt, :], op0=ALU.mult, op1=ALU.add)
                    sumexp = stats[:, mt, 0:1]
                    rsum = stats[:, mt, 1:2]
                    nc.vector.tensor_reduce(sumexp, exps[:, mt, :],
                                            axis=AX.X, op=ALU.add)
                    nc.vector.reciprocal(rsum, sumexp)
                    nc.vector.tensor_scalar_mul(ctx_t[:, mt, :],
                                                ctx_t[:, mt, :], rsum)
        st.update(EH=EH, scores=scores, exps=exps, ctx_t=ctx_t)

    def p45(c):
        # context transpose to [feature, row] for fc1; the softmax 1/sumexp
        # normalization rides along for free as a diag matrix in place of
        # the transpose identity
        st = S[c]
        ctx_t = st["ctx_t"]
        ctxT = t8tile([P, KT, CH], "ctxT")
        for mt in range(MT):
            for g in range(2):
                trp = pstr("trc", dtype=BF16)
                for ft in range(4):
                    nc.tensor.transpose(
                        trp[:, ft * P:(ft + 1) * P],
                        ctx_t[:, mt, (g * 4 + ft) * P:(g * 4 + ft + 1) * P],
                        identity_bf)
                nc.vector.tensor_copy(
                    ctxT[:, g * 4:(g + 1) * 4, mt * P:(mt + 1) * P],
                    trp.rearrange("p (ft x) -> p ft x", ft=4))
        st["ctxT"] = ctxT

    def p6(c):
        st = S[c]
        x1T = t8tile([P, KT, CH], "x1T")
        pq = psq("pq")
        for mh in range(2):
            w1 = wtile([P, KT, 512], "w1t")
            nc.sync.dma_start(
                w1, io["W1"][mh].rearrange("p (ko f) -> p ko f", ko=KT))
            for mi in range(4):
                m = mh * 4 + mi
                pm = psmm("pm_fc1")
                for kt in range(KT):
                    nc.tensor.matmul(pm, lhsT=w1[:, kt, mi * P:(mi + 1) * P],
                                     rhs=st["ctxT"][:, kt, :],
                                     start=(kt == 0), stop=(kt == KT - 1))
                nc.scalar.activation(x1T[:, m, :], pm, AF.Relu,
                                     bias=b1_pp[:, m:m + 1])
            # fc2 rides behind each half's evictions so the final output
            # eviction starts right after the last fc1 matmul
            for kt in range(mh * 4, mh * 4 + 4):
                nc.tensor.matmul(pq, lhsT=W2sb[:, kt:kt + 1],
                                 rhs=x1T[:, kt, :],
                                 start=(kt == 0), stop=(kt == KT - 1))
        st["pq"] = pq

    def p7(c):
        st = S[c]
        r0 = c * CH
        q_rowT = acts.tile([1, CH], F32, tag="q_rowT", bufs=1, name="q_rowT")
        nc.scalar.activation(q_rowT[0:1, :], st["pq"][0:1, :], AF.Identity,
                             bias=b2sb[0:1, 0:1])
        nc.sync.dma_start(q_ap[r0:r0 + CH, 0][None, :], q_rowT[0:1, :])

    STAGE_FNS = {"binit": lambda c: binit(), "p1": p1, "p1b": p1b,
                 "p2a": p2a, "p2b": p2b, "p3": p3, "p45": p45, "p6": p6,
                 "p7": p7}

    # Emission order: chunk 1's enc GEMMs are emitted inside chunk 0's
    # softmax tail so the PE stays busy while the DVE finishes the last
    # head / normalization.
    order = [(0, "binit"), (0, "p1"), (1, "p1"),
             (0, "p1b"), (0, "p2a"), (0, "p2b"), (0, "p3"),
             (1, "p1b"),
             (0, "p45"), (0, "p6"), (0, "p7"),
             (1, "p2a"), (1, "p2b"), (1, "p3"),
             (1, "p45"), (1, "p6"), (1, "p7")]

    for c, nm in order:
        sc_(c, nm)
        STAGE_FNS[nm](c)
    sc_(0, None)


_NC_CACHE = None


def build():
    global _NC_CACHE
    if _NC_CACHE is not None:
        return _NC_CACHE
    nc = bacc.Bacc(trn_type="TRN2", target_bir_lowering=False, debug=False,
                   enable_asserts=False)
    io = {}
    # all tensors are host-pre-arranged so every DMA is contiguous per
    # partition (8KB descriptors): W[.., p, ko*f] = W_orig[ko*128+p, f]
    shapes_bf = {
        "pk": [P, 20488],
        "W_eh": [2, P, KT * 512],
        "W_heads": [NH, 2, P, KT * 512],
        "W1": [2, P, KT * 512],
    }
    shapes_f32 = {
        "bias_pp": [P, 3 * KT], "b2": [1],
    }
    for name, shp in shapes_bf.items():
        io[name] = nc.dram_tensor(name, shp, BF16, kind="ExternalInput").ap()
    for name, shp in shapes_f32.items():
        io[name] = nc.dram_tensor(name, shp, F32, kind="ExternalInput").ap()
    io["q"] = nc.dram_tensor("q", [RPC, 1], F32, kind="ExternalOutput").ap()

    from contextlib import ExitStack
    with tile.TileContext(nc) as tc, ExitStack() as ctx:
        _body(nc, tc, io, ctx)
    nc.compile()
    _NC_CACHE = nc
    return nc


def _ktile(w, nhalves):
    # [K, N] -> [nhalves, 128, KT*(N/nhalves)]: w_r[nh, p, ko*f] =
    # w[ko*128+p, nh*(N/nhalves)+f]
    K, N = w.shape
    nh = N // nhalves
    r = w.reshape(K // P, P, nhalves, nh).transpose(2, 1, 0, 3)
    return np.ascontiguousarray(r.reshape(nhalves, P, (K // P) * nh))


def _prep_inputs(inputs):
    import ml_dtypes
    bf16 = ml_dtypes.bfloat16
    arrs = {k: np.ascontiguousarray(np.asarray(v, dtype=np.float32))
            for k, v in inputs.items()}
    # host-side weight folding: dec_input feeds only decoder_H (no relu in
    # between), so W_fused = W_dec_in @ W_dh, b_fused = b_dec_in @ W_dh + b_dh
    wf = arrs["W_dec_in"] @ arrs["W_dh"]
    bfu = arrs["b_dec_in"] @ arrs["W_dh"] + arrs["b_dh"]
    wfa = np.ascontiguousarray(
        np.concatenate([wf, bfu[None, :]], axis=0)).astype(bf16)
    # s^T tiled as [128, 4*B]: sT[p, kt*B+r] = s[r, kt*128+p]
    sT = np.ascontiguousarray(
        arrs["s"].T.reshape(4, P, B).transpose(1, 0, 2)).astype(bf16)
    aT = np.ascontiguousarray(                # [129, B]: ones row appended
        np.concatenate([arrs["a"].T, np.ones((1, B), np.float32)],
                       axis=0)).astype(bf16)
    wcast = {
        "W_eh": _ktile(arrs["W_eh"], 2).astype(bf16),
        "W_heads": np.ascontiguousarray(np.stack(
            [_ktile(arrs["W_heads"][h], 2) for h in range(NH)])).astype(bf16),
        "W1": _ktile(arrs["W1"], 2).astype(bf16),
    }
    wenc_t = _ktile(arrs["W_enc_in"][0:512], 1)[0].astype(bf16)  # [P, 4096]
    wencr = arrs["W_enc_in"][512:544].astype(bf16)               # [32, 1024]
    bcast = {
        "bias_pp": np.ascontiguousarray(np.concatenate(
            [arrs[k].reshape(KT, P).T for k in ("b_enc_in", "b_eh", "b1")],
            axis=1)),
        "b2": arrs["b2"],
    }
    in_maps = []
    for c in range(NCORES):
        m = dict(bcast)
        m.update(wcast)
        # packed startup tensor: [wenc | sT_c0 | wencr | aown_c0 | aoth_c0 |
        # sT_c1 | aown_c1 | aoth_c1 | W2 | Wfa | b_heads]
        pkm = np.zeros((P, 20488), bf16)
        pkm[:, 0:4096] = wenc_t
        pkm[0:ENC_REM, 6144:7168] = wencr
        for ch in range(2):
            r0 = c * RPC + ch * CH
            o = 4096 if ch == 0 else 8192
            oa = 7168 if ch == 0 else 10240
            pkm[:, o:o + 2048] = sT[:, :, r0:r0 + CH].reshape(P, 4 * CH)
            aTc = aT[:, r0:r0 + CH]
            pkm[0:ACTD, oa:oa + CH] = aTc[0:ACTD]
            pkm[0:DEC_IN + 1, oa + CH:oa + 2 * CH] = aTc[ACTD:P + 1]
        pkm[:, 11264:11272] = arrs["W2"].reshape(KT, P).T.astype(bf16)
        pkm[0:DEC_IN + 1, 11272:12296] = wfa
        pkm[0:1, 12296:20488] = arrs["b_heads"].reshape(1, -1).astype(bf16)
        m["pk"] = pkm
        in_maps.append(m)
    return in_maps


def run(inputs, trace=False):
    from concourse.bass_utils import run_bass_kernel_spmd
    nc = build()
    in_maps = _prep_inputs(inputs)
    res = run_bass_kernel_spmd(nc, in_maps, core_ids=list(range(NCORES)),
                               trace=trace)
    q = np.concatenate([r["q"] for r in res.results], axis=0)
    return np.ascontiguousarray(q.astype(np.float32)), res


def kernel(**inputs) -> np.ndarray:
    q, _ = run(inputs, trace=False)
    return q
